# revision 14
# baseline (speedup 1.0000x reference)
"""Trainium2 Bass kernel for nn_Attention_Module (dense_transformer).

Data-parallel over batch: B=64 split across 8 NeuronCores (8 per core).
Per core, activations are channel-major [C, tokens] with the 8 local
batches' 320 tokens reordered host-side into a z-block tile (8*64=512
template tokens) + 4 x-block tiles (2 batches x 256 search tokens each).

v3: bf16 activations/weights (PSUM accumulation fp32); single scalar
activation-table set (ln+exp; rsqrt = exp(-0.5 ln x)); transposed
attention scores G^T = kT.T @ qT with the k-side row norm folded into
the Exp's per-partition scale and one full 128x128 Exp per
(branch, group); softmax denominator via a ones column interleaved
into v (one matmul emits AV and the row sum); per-head 32x32
tile_position-packed AV; residuals accumulated into PSUM via identity
matmuls; W_up@W_end folded host-side into W_comb; fully contiguous
host-reordered DMA; interleaved front/back emission for PE density.

Self-contained: only imports infra from /opt/trn_rl_repo.
"""
import sys

sys.path.insert(0, "/opt/trn_rl_repo")

from contextlib import ExitStack

import numpy as np

import bass_rust as _bass_rust
import concourse.bacc as bacc
import concourse.tile as tile
from concourse import mybir
from concourse.hw_specs import get_activation_tables

F32 = mybir.dt.float32
BF16 = mybir.dt.bfloat16
F8 = mybir.dt.float8e4
SC = 64.0  # fp8 weight pre-scale (folded back out via activation scales)
AF = mybir.ActivationFunctionType
OP = mybir.AluOpType
AX = mybir.AxisListType

B_LOC = 8          # batches per core
DIM = 512
HID = 256
HEADS = 8
NZ, NX = 64, 256   # template / search tokens per batch
NTOK = NZ + NX     # 320
NT = 5             # token tiles of 512
EPS_LN = 1e-5
TINY = 1e-24       # guards ln of exact-zero row norms
VEXT = 528         # per-group v_ext row: max(8*65, 2*257) padded


def _bbs(j):
    """Branch segments inside token-tile j: list of (col_off, width)."""
    if j == 0:
        return [(64 * b, 64) for b in range(B_LOC)]
    return [(0, 256), (256, 256)]


def _chunks(off, w):
    """Token-partition chunks (tb, part_off, part_w) for a branch segment."""
    if w == 64:
        return [(off // 128, off % 128, 64)]
    return [(off // 128, 0, 128), (off // 128 + 1, 0, 128)]


def build_nc():
    nc = bacc.Bacc("TRN2", target_bir_lowering=False, debug=False,
                   num_devices=8)

    # ---- DRAM I/O (per-core shapes, host pre-reordered / pre-cast) ----
    x1_e = nc.declare_dram_parameter("x1", [NT, 4, 128, 512], BF16, isOutput=False)
    x18_e = nc.declare_dram_parameter("x1_8", [NT, 4, 128, 512], F8, isOutput=False)
    x28_e = nc.declare_dram_parameter("x2_8", [NT, 4, 128, 512], F8, isOutput=False)
    out_e = nc.declare_dram_parameter("out", [NT, 4, 128, 512], BF16, isOutput=True)
    wlin8_e = nc.declare_dram_parameter("W_lin8", [2, 128, 2, 2 * DIM], F8, isOutput=False)
    wdown_e = nc.declare_dram_parameter("W_down", [4, 128, HID], BF16, isOutput=False)
    wq_e = nc.declare_dram_parameter("WqT", [2, 128, HID], BF16, isOutput=False)
    wk_e = nc.declare_dram_parameter("WkT", [2, 128, HID], BF16, isOutput=False)
    wv_e = nc.declare_dram_parameter("WvT", [2, 128, HID], BF16, isOutput=False)
    wo_e = nc.declare_dram_parameter("WoT", [2, 128, HID], BF16, isOutput=False)
    wend8_e = nc.declare_dram_parameter("W_end8", [2, 128, 2, DIM], F8, isOutput=False)
    wcomb_e = nc.declare_dram_parameter("W_comb", [2, 128, DIM], BF16, isOutput=False)
    blin_e = nc.declare_dram_parameter("b_lin_pg", [128, 8], F32, isOutput=False)
    bdown_e = nc.declare_dram_parameter("b_down_pg", [128, 2], F32, isOutput=False)
    bend_e = nc.declare_dram_parameter("b_end_pg", [128, 4], F32, isOutput=False)
    gamma_e = nc.declare_dram_parameter("gamma_pg", [128, 4], F32, isOutput=False)
    beta_e = nc.declare_dram_parameter("beta_pg", [128, 4], F32, isOutput=False)
    temp_e = nc.declare_dram_parameter("lntau_col", [128, 2], F32, isOutput=False)
    ones_e = nc.declare_dram_parameter("ones_in", [128, 128], BF16, isOutput=False)
    ident_e = nc.declare_dram_parameter("ident_in", [128, 128], BF16, isOutput=False)
    id64_e = nc.declare_dram_parameter("id64_in", [128, 128], BF16, isOutput=False)

    with tile.TileContext(nc) as tc, ExitStack() as ctx:
        wts = ctx.enter_context(tc.tile_pool(name="wts", bufs=1))
        xload = ctx.enter_context(tc.tile_pool(name="xload", bufs=3))
        u1p = ctx.enter_context(tc.tile_pool(name="u1p", bufs=1))
        rp = ctx.enter_context(tc.tile_pool(name="rp", bufs=2))
        u2p = ctx.enter_context(tc.tile_pool(name="u2p", bufs=1))
        ap_ = ctx.enter_context(tc.tile_pool(name="ap", bufs=2))
        bqp = ctx.enter_context(tc.tile_pool(name="bqp", bufs=1))
        qkvp = ctx.enter_context(tc.tile_pool(name="qkvp", bufs=2))
        sqp = ctx.enter_context(tc.tile_pool(name="sqp", bufs=2))
        nrmp = ctx.enter_context(tc.tile_pool(name="nrmp", bufs=2))
        qtp = ctx.enter_context(tc.tile_pool(name="qtp", bufs=2))
        etp = ctx.enter_context(tc.tile_pool(name="etp", bufs=3))
        rp2 = ctx.enter_context(tc.tile_pool(name="rp2", bufs=2))
        avp = ctx.enter_context(tc.tile_pool(name="avp", bufs=1))
        o1p = ctx.enter_context(tc.tile_pool(name="o1p", bufs=1))
        scr = ctx.enter_context(tc.tile_pool(name="scr", bufs=2))
        prep = ctx.enter_context(tc.tile_pool(name="prep", bufs=1))
        statp = ctx.enter_context(tc.tile_pool(name="statp", bufs=1))
        outp = ctx.enter_context(tc.tile_pool(name="outp", bufs=1))
        ps = ctx.enter_context(tc.tile_pool(name="ps", bufs=2, space="PSUM"))
        pst = ctx.enter_context(tc.tile_pool(name="pst", bufs=2, space="PSUM"))
        psg = ctx.enter_context(tc.tile_pool(name="psg", bufs=2, space="PSUM"))
        psav = ctx.enter_context(tc.tile_pool(name="psav", bufs=2, space="PSUM"))

        # ---- weights / constants in SBUF ----
        wlin8_sb = wts.tile([128, 2, 2, 2 * DIM], F8)
        wdown_sb = wts.tile([128, 4, HID], BF16)
        wq_sb = wts.tile([128, 2, HID], BF16)
        wk_sb = wts.tile([128, 2, HID], BF16)
        wv_sb = wts.tile([128, 2, HID], BF16)
        wo_sb = wts.tile([128, 2, HID], BF16)
        wend8_sb = wts.tile([128, 2, 2, DIM], F8)
        wcomb_sb = wts.tile([128, 2, DIM], BF16)

        blin_sb = wts.tile([128, 8], F32)
        bdown_sb = wts.tile([128, 2], F32)
        bend_sb = wts.tile([128, 4], F32)
        gamma_sb = wts.tile([128, 4], F32)
        beta_sb = wts.tile([128, 4], F32)
        tempc_sb = wts.tile([128, 2], F32)
        ones_sb = wts.tile([128, 128], BF16)
        ident_sb = wts.tile([128, 128], BF16)
        id64_sb = wts.tile([128, 128], BF16)
        tiny_sb = wts.tile([128, 1], F32)
        nc.vector.memset(tiny_sb[:], TINY)
        epsln_sb = wts.tile([128, 1], F32)
        nc.vector.memset(epsln_sb[:], EPS_LN)
        zero_sb = wts.tile([128, 1], F32)
        nc.vector.memset(zero_sb[:], 0.0)

        def emit_loads(j):
            x1t8 = xload.tile([128, 4, 512], F8, tag="x1l8")
            x2t8 = xload.tile([128, 4, 512], F8, tag="x2l8")
            x1t = xload.tile([128, 4, 512], BF16, tag="x1l")
            nc.sync.dma_start(x1t8[:], x18_e[j].rearrange("k p t -> p k t"))
            nc.sync.dma_start(x2t8[:], x28_e[j].rearrange("k p t -> p k t"))
            nc.sync.dma_start(x1t[:], x1_e[j].rearrange("k p t -> p k t"))
            return (x1t, x1t8, x2t8)

        def emit_front_a(j, ld):
            x1t, x1t8, x2t8 = ld
            # ---- S1: h1 = relu((W_lin8^T X1_8)/SC + b); r = y1 + u1 (fp8) ----
            u1 = u1p.tile([128, 4, 512], BF16)
            r = rp.tile([128, 4, 512], F8)
            for m in [4, 5, 6, 7, 0, 1, 2, 3]:
                pt = ps.tile([128, 512], F32, tag="ps")
                for kt in range(2):
                    nc.tensor.matmul(pt[:], wlin8_sb[:, kt, :, 128 * m:128 * (m + 1)],
                                     x1t8[:, 2 * kt:2 * kt + 2, :],
                                     start=(kt == 0), stop=(kt == 1),
                                     perf_mode=mybir.MatmulPerfMode.DoubleRow)
                if m >= 4:
                    nc.scalar.activation(u1[:, m - 4, :], pt[:], AF.Relu,
                                         bias=blin_sb[:, m:m + 1], scale=1.0 / SC)
                else:
                    ytmp = scr.tile([128, 512], BF16, tag="ytmp")
                    nc.scalar.activation(ytmp[:], pt[:], AF.Relu,
                                         bias=blin_sb[:, m:m + 1], scale=1.0 / SC)
                    nc.gpsimd.tensor_add(r[:, m, :], ytmp[:], u1[:, m, :])
            return dict(x1t=x1t, x2t8=x2t8, u1=u1, r=r)

        def emit_front_s1b(j, st):
            x2t8 = st["x2t8"]
            # ---- S1b: u2 = relu(W_lin[:,512:]^T X2 + b2) ----
            u2 = u2p.tile([128, 4, 512], BF16)
            for m in range(4):
                pt = ps.tile([128, 512], F32, tag="ps")
                for kt in range(2):
                    nc.tensor.matmul(
                        pt[:],
                        wlin8_sb[:, kt, :, 512 + 128 * m:512 + 128 * (m + 1)],
                        x2t8[:, 2 * kt:2 * kt + 2, :],
                        start=(kt == 0), stop=(kt == 1),
                        perf_mode=mybir.MatmulPerfMode.DoubleRow)
                nc.scalar.activation(u2[:, m, :], pt[:], AF.Relu,
                                     bias=blin_sb[:, 4 + m:5 + m], scale=1.0 / SC)
            st["u2"] = u2

        def emit_front_b(j, st):
            u1, u2 = st["u1"], st["u2"]
            bbs = _bbs(j)
            nb = len(bbs)
            w_ = bbs[0][1]

            # ---- S2: A = relu(W_down^T u1 + b_down); Bq likewise from u2 ----
            A = ap_.tile([128, 2, 512], BF16)
            Bq = bqp.tile([128, 2, 512], BF16)
            for (dst, src) in ((A, u1), (Bq, u2)):
                for m in range(2):
                    pt = ps.tile([128, 512], F32, tag="ps")
                    for kt in range(4):
                        nc.tensor.matmul(pt[:],
                                         wdown_sb[:, kt, 128 * m:128 * (m + 1)],
                                         src[:, kt, :],
                                         start=(kt == 0), stop=(kt == 3))
                    nc.scalar.activation(dst[:, m, :], pt[:], AF.Relu,
                                         bias=bdown_sb[:, m:m + 1])

            # ---- S3: q = Wq@Bq, k = Wk@A (channel-major, SBUF copies);
            #          v -> v_ext with a ones column per branch segment ----
            q = qkvp.tile([128, 2, 512], BF16, tag="q")
            k = qkvp.tile([128, 2, 512], BF16, tag="k")
            vx = qkvp.tile([128, 2, VEXT], BF16, tag="vx")
            for (dst, w_sb, src) in ((q, wq_sb, Bq), (k, wk_sb, A)):
                for m in range(2):
                    pt = ps.tile([128, 512], F32, tag="ps")
                    for kt in range(2):
                        nc.tensor.matmul(pt[:], w_sb[:, kt, 128 * m:128 * (m + 1)],
                                         src[:, kt, :],
                                         start=(kt == 0), stop=(kt == 1))
                    nc.vector.tensor_copy(dst[:, m, :], pt[:])
            for m in range(2):
                pt = ps.tile([128, 512], F32, tag="ps")
                for kt in range(2):
                    nc.tensor.matmul(pt[:], wv_sb[:, kt, 128 * m:128 * (m + 1)],
                                     A[:, kt, :], start=(kt == 0), stop=(kt == 1))
                vxg = vx[:, m, 0:nb * (w_ + 1)].rearrange(
                    "p (n e) -> p n e", e=w_ + 1)
                nc.vector.memset(vxg[:, :, w_:w_ + 1], 1.0)
                nc.vector.tensor_copy(
                    vxg[:, :, 0:w_],
                    pt[:].rearrange("p (n w) -> p n w", w=w_))
            # kT transpose needs no norm scaling: do it as soon as k lands
            kT = qtp.tile([128, 4, 256], BF16, tag="kT")
            for tb in range(4):
                pt = pst.tile([128, 256], BF16, tag="pst")
                for g in range(2):
                    nc.tensor.matmul(
                        pt[:, 128 * g:128 * (g + 1)],
                        k[:, g, 128 * tb:128 * (tb + 1)], ident_sb[:],
                        is_transpose=True, start=(g == 0), stop=(g == 1))
                nc.vector.tensor_copy(kT[:, tb, :], pt[:])
            return dict(A=A, q=q, k=k, vx=vx, kT=kT)

        def emit_back_a(j, st):
            bbs = _bbs(j)
            nb = len(bbs)
            q, k = st["q"], st["k"]
            # ---- S4: rsqrt(row L2 norms) = exp(-0.5 ln(ssq)) ----
            w = 512 // nb
            rn = {}
            for (name, t_) in (("q", q), ("k", k)):
                sq = sqp.tile([128, 2, 512], BF16, tag="sq")
                nc.vector.tensor_mul(sq[:], t_[:], t_[:])
                ssq = nrmp.tile([128, 2, nb], F32, tag="ssq" + name)
                nc.vector.reduce_sum(
                    ssq[:], sq[:].rearrange("p g (n w) -> p g n w", w=w), axis=AX.X)
                lnt = nrmp.tile([128, 2, nb], F32, tag="ln" + name)
                nc.scalar.activation(lnt[:], ssq[:], AF.Ln, bias=tiny_sb[:, 0:1])
                rr = nrmp.tile([128, 2, nb], F32, tag="rn" + name)
                if name == "q":  # fold per-head temperature: exp(-ln(ssq)/2 + ln tau)
                    for g in range(2):
                        nc.scalar.activation(rr[:, g, :], lnt[:, g, :], AF.Exp,
                                             scale=-0.5,
                                             bias=tempc_sb[:, g:g + 1])
                else:
                    nc.scalar.activation(rr[:], lnt[:], AF.Exp, scale=-0.5)
                rn[name] = rr
            for g in range(2):
                for bi, (off, w_) in enumerate(bbs):
                    nc.vector.tensor_scalar_mul(
                        q[:, g, off:off + w_], in0=q[:, g, off:off + w_],
                        scalar1=rn["q"][:, g, bi:bi + 1])
            st["rnk"] = rn["k"]

        def emit_back_attn(j, st):
            bbs = _bbs(j)
            nb = len(bbs)
            q, vx, rnk, kT = st["q"], st["vx"], st["rnk"], st["kT"]
            # ---- S5: PE-transpose scaled q -> token-major qT ----
            qT = qtp.tile([128, 4, 256], BF16, tag="qT")
            for tb in range(4):
                pt = pst.tile([128, 256], BF16, tag="pst")
                for g in range(2):
                    nc.tensor.matmul(
                        pt[:, 128 * g:128 * (g + 1)],
                        q[:, g, 128 * tb:128 * (tb + 1)], ident_sb[:],
                        is_transpose=True, start=(g == 0), stop=(g == 1))
                nc.vector.tensor_copy(qT[:, tb, :], pt[:])

            # ---- S6-S8: per (branch, group): G^T -> exp(scale=rn_k) ->
            #      AV matmul with interleaved ones col -> R=1/S -> scale ----
            av = avp.tile([128, 2, 512], BF16)
            for bi, (off, w_) in enumerate(bbs):
                chunks = _chunks(off, w_)
                for g in range(2):
                    gps = psg.tile([128, 128], F32, tag="gps")
                    for ci, (tb, tpo, cw) in enumerate(chunks):
                        nc.tensor.matmul(
                            gps[:],
                            kT[tpo:tpo + cw, tb, 128 * g:128 * (g + 1)],
                            qT[tpo:tpo + cw, tb, 128 * g:128 * (g + 1)],
                            start=(ci == 0), stop=(ci == len(chunks) - 1))
                    ET = etp.tile([128, 128], BF16, tag="et")
                    nc.scalar.activation(ET[:], gps[:], AF.Exp,
                                         bias=zero_sb[:, 0:1],
                                         scale=rnk[:, g, bi:bi + 1])
                    pav = psav.tile([128, 512], F32, tag="pav")
                    e1 = w_ + 1
                    for h in range(4):
                        hs = slice(32 * h, 32 * (h + 1))
                        nc.tensor.matmul(
                            pav[hs, 0:e1], ET[hs, hs],
                            vx[hs, g, bi * e1:(bi + 1) * e1],
                            start=True, stop=True, tile_position=(32 * h, 32 * h))
                    R = rp2.tile([128, 1], F32, tag="R")
                    nc.vector.reciprocal_approx_fast(R[:], pav[:, w_:w_ + 1])
                    nc.scalar.mul(av[:, g, off:off + w_], pav[:, 0:w_],
                                  mul=R[:, 0:1])

            st["av"] = av

        def emit_back_tail(j, st):
            x1t, r, A, av = st["x1t"], st["r"], st["A"], st["av"]
            # ---- S9: o1 = Wo@av + A (A added via identity matmul) ----
            o1 = o1p.tile([128, 2, 512], BF16)
            for m in range(2):
                pt = ps.tile([128, 512], F32, tag="ps")
                for kt in range(2):
                    nc.tensor.matmul(pt[:], wo_sb[:, kt, 128 * m:128 * (m + 1)],
                                     av[:, kt, :], start=(kt == 0), stop=False)
                nc.tensor.matmul(pt[:], ident_sb[:], A[:, m, :],
                                 start=False, stop=True)
                nc.vector.tensor_copy(o1[:, m, :], pt[:])

            # ---- S10/S11: pre = W_end^T r + W_comb^T o1 + t1 + b_eff ----
            pre = prep.tile([128, 4, 512], BF16)
            s1ps = psav.tile([128, 512], F32, tag="pav")
            s2ps = psav.tile([128, 512], F32, tag="pav")
            for m in range(4):
                pt = ps.tile([128, 512], F32, tag="ps")
                for kt in range(2):
                    nc.tensor.matmul(pt[:], wend8_sb[:, kt, :, 128 * m:128 * (m + 1)],
                                     r[:, 2 * kt:2 * kt + 2, :],
                                     start=(kt == 0), stop=False,
                                     perf_mode=mybir.MatmulPerfMode.DoubleRow)
                for kt in range(2):
                    nc.tensor.matmul(pt[:], wcomb_sb[:, kt, 128 * m:128 * (m + 1)],
                                     o1[:, kt, :], start=False, stop=False)
                nc.tensor.matmul(pt[:], id64_sb[:], x1t[:, m, :],
                                 start=False, stop=True)
                nc.vector.tensor_scalar(pre[:, m, :], in0=pt[:],
                                        scalar1=1.0 / SC,
                                        scalar2=bend_sb[:, m:m + 1],
                                        op0=OP.mult, op1=OP.add)
                p2 = scr.tile([128, 512], BF16, tag="p2")
                nc.scalar.activation(p2[:], pre[:, m, :], AF.Square)
                nc.tensor.matmul(s1ps[:], ones_sb[:], pre[:, m, :],
                                 start=(m == 0), stop=(m == 3))
                nc.tensor.matmul(s2ps[:], ones_sb[:], p2[:],
                                 start=(m == 0), stop=(m == 3))

            # ---- S12: mu/rstd (rows replicated); rstd = exp(-0.5 ln(var)) ----
            mu = statp.tile([128, 512], BF16, tag="mu")
            nc.vector.tensor_scalar_mul(mu[:], in0=s1ps[:], scalar1=1.0 / DIM)
            msq = statp.tile([128, 512], BF16, tag="msq")
            nc.vector.tensor_mul(msq[:], mu[:], mu[:])
            var = statp.tile([128, 512], F32, tag="var")
            nc.vector.scalar_tensor_tensor(var[:], in0=s2ps[:], scalar=1.0 / DIM,
                                           in1=msq[:], op0=OP.mult, op1=OP.subtract)
            lnv = statp.tile([128, 512], F32, tag="lnv")
            nc.scalar.activation(lnv[:], var[:], AF.Ln, bias=epsln_sb[:, 0:1])
            rstd = statp.tile([128, 512], BF16, tag="rstd")
            nc.scalar.activation(rstd[:], lnv[:], AF.Exp, scale=-0.5)

            # ---- S13: out = ((pre - mu) * rstd) * gamma + beta ----
            ot = outp.tile([128, 4, 512], BF16)
            for m in range(4):
                t1 = scr.tile([128, 512], BF16, tag="t1")
                nc.gpsimd.tensor_sub(t1[:], pre[:, m, :], mu[:])
                mgb = scr.tile([128, 512], BF16, tag="mgb")
                nc.vector.tensor_mul(mgb[:], t1[:], rstd[:])
                nc.vector.tensor_scalar(
                    ot[:, m, :], in0=mgb[:], scalar1=gamma_sb[:, m:m + 1],
                    scalar2=beta_sb[:, m:m + 1], op0=OP.mult, op1=OP.add)

            # ---- S14: store ----
            nc.sync.dma_start(out_e[j].rearrange("k p t -> p k t"), ot[:])

        order = [1, 2, 0, 3, 4]
        nc.scalar.dma_start(wlin8_sb[:, 0], wlin8_e[0].rearrange("p o m -> p o m"))
        nc.scalar.dma_start(blin_sb[:], blin_e[:, :])
        nc.gpsimd.dma_start(wlin8_sb[:, 1], wlin8_e[1].rearrange("p o m -> p o m"))
        ld = emit_loads(order[0])
        nc.gpsimd.dma_start(wdown_sb[:], wdown_e.rearrange("k p m -> p k m"))
        nc.sync.dma_start(wq_sb[:], wq_e.rearrange("k p m -> p k m"))
        nc.sync.dma_start(wk_sb[:], wk_e.rearrange("k p m -> p k m"))
        nc.sync.dma_start(wv_sb[:], wv_e.rearrange("k p m -> p k m"))
        nc.sync.dma_start(wo_sb[:], wo_e.rearrange("k p m -> p k m"))
        nc.sync.dma_start(wend8_sb[:], wend8_e.rearrange("k p o m -> p k o m"))
        nc.sync.dma_start(wcomb_sb[:], wcomb_e.rearrange("k p m -> p k m"))
        nc.sync.dma_start(bdown_sb[:], bdown_e[:, :])
        nc.sync.dma_start(bend_sb[:], bend_e[:, :])
        nc.sync.dma_start(gamma_sb[:], gamma_e[:, :])
        nc.sync.dma_start(beta_sb[:], beta_e[:, :])
        nc.sync.dma_start(tempc_sb[:], temp_e[:, :])
        nc.sync.dma_start(ones_sb[:], ones_e[:, :])
        nc.sync.dma_start(ident_sb[:], ident_e[:, :])
        nc.sync.dma_start(id64_sb[:], id64_e[:, :])

        st = emit_front_a(order[0], ld)
        emit_front_s1b(order[0], st)
        st.update(emit_front_b(order[0], st))
        emit_back_a(order[0], st)
        prev = (order[0], st)
        for j in order[1:]:
            ld = emit_loads(j)
            st = emit_front_a(j, ld)
            emit_front_s1b(j, st)
            emit_back_attn(prev[0], prev[1])
            st.update(emit_front_b(j, st))
            emit_back_a(j, st)
            emit_back_tail(prev[0], prev[1])
            prev = (j, st)
        emit_back_attn(prev[0], prev[1])
        emit_back_tail(prev[0], prev[1])

    # Run the act-table insertion pass with a curated set list so every
    # activation (relu/ln/exp/square/copy) resolves to the one combined
    # natural_log_exp_and_others set -> a single ACT_TABLE_LOAD.
    shared = {AF.Exp, AF.Ln, AF.Relu, AF.Square, AF.Copy, AF.Identity}
    tabs = get_activation_tables(nc.m.arch)
    curated = []
    for name, fns in tabs.items():
        if name != "natural_log_exp_and_others":
            fns = fns - shared
        curated.append((name, fns))

    orig = bacc.Bacc.insert_act_table_loads

    def _curated(self):
        _bass_rust.insert_act_table_loads(self, curated)

    nc.insert_act_table_loads = _curated.__get__(nc)
    try:
        nc.compile()
    finally:
        nc.insert_act_table_loads = orig.__get__(nc)
    return nc


# ---------------- host side ----------------
_CACHE = {}


def _get_runner():
    if "runner" in _CACHE:
        return _CACHE["runner"]
    import jax
    from jax.sharding import Mesh, PartitionSpec
    from jax.experimental.shard_map import shard_map
    from concourse.bass2jax import (
        _bass_exec_p, install_neuronx_cc_hook, partition_id_tensor)
    import concourse.mybir as mybir_

    nc = build_nc()
    install_neuronx_cc_hook()
    partition_name = nc.partition_id_tensor.name if nc.partition_id_tensor else None
    in_names, out_names, out_avals, zero_outs = [], [], [], []
    for alloc in nc.m.functions[0].allocations:
        if not isinstance(alloc, mybir_.MemoryLocationSet):
            continue
        name = alloc.memorylocations[0].name
        if alloc.kind == "ExternalInput":
            if name != partition_name:
                in_names.append(name)
        elif alloc.kind == "ExternalOutput":
            out_names.append(name)
            shape = tuple(alloc.tensor_shape)
            dtype = mybir_.dt.np(alloc.dtype)
            out_avals.append(jax.core.ShapedArray(shape, dtype))
            zero_outs.append(np.zeros(shape, dtype))
    n_params, n_outs = len(in_names), len(out_avals)
    all_in = list(in_names) + list(out_names)
    if partition_name is not None:
        all_in.append(partition_name)
    donate = tuple(range(n_params, n_params + n_outs))

    def _body(*args):
        operands = list(args)
        if partition_name is not None:
            operands.append(partition_id_tensor())
        return tuple(_bass_exec_p.bind(
            *operands, out_avals=tuple(out_avals), in_names=tuple(all_in),
            out_names=tuple(out_names), lowering_input_output_aliases=(),
            sim_require_finite=True, sim_require_nnan=True, nc=nc))

    devices = jax.devices()[:8]
    mesh = Mesh(np.asarray(devices), ("core",))
    fn = jax.jit(
        shard_map(_body, mesh=mesh,
                  in_specs=(PartitionSpec("core"),) * (n_params + n_outs),
                  out_specs=(PartitionSpec("core"),) * n_outs,
                  check_rep=False),
        donate_argnums=donate, keep_unused=True)
    _CACHE["runner"] = (fn, in_names, out_names, out_avals, zero_outs)
    return _CACHE["runner"]


def _reorder_x(xc, BF):
    """[8, 512, 320] fp32 -> [5, 4, 128, 512] bf16, token-reordered."""
    dev = np.empty((NT, 4, 128, 512), dtype=BF)
    z = np.transpose(xc[:, :, 0:64], (1, 0, 2)).reshape(512, 512)
    dev[0] = z.reshape(4, 128, 512).astype(BF)
    for j in range(1, NT):
        xx = np.transpose(xc[2 * j - 2:2 * j, :, 64:320], (1, 0, 2))
        dev[j] = xx.reshape(512, 512).reshape(4, 128, 512).astype(BF)
    return dev


def _unreorder_out(dev):
    """[5, 4, 128, 512] (any float) -> [8, 512, 320] fp32."""
    out = np.empty((B_LOC, DIM, NTOK), np.float32)
    z = dev[0].astype(np.float32).reshape(512, 8, 64)
    out[:, :, 0:64] = np.transpose(z, (1, 0, 2))
    for j in range(1, NT):
        xx = dev[j].astype(np.float32).reshape(512, 2, 256)
        out[2 * j - 2:2 * j, :, 64:320] = np.transpose(xx, (1, 0, 2))
    return out


def _prep_inputs(inputs):
    import ml_dtypes
    BF = ml_dtypes.bfloat16
    f = lambda a: np.ascontiguousarray(np.asarray(a), dtype=np.float32)
    x1 = f(inputs["x1"]).reshape(64, DIM, NTOK)
    x2 = f(inputs["x2"]).reshape(64, DIM, NTOK)
    temp = np.log(f(inputs["temperature"]).reshape(HEADS))
    temp_col = np.empty((128, 2), np.float32)
    for g in range(2):
        for hh in range(4):
            temp_col[32 * hh:32 * (hh + 1), g] = temp[4 * g + hh]

    F8N = ml_dtypes.float8_e4m3
    W_up = f(inputs["W_up"])
    W_end = f(inputs["W_end"])
    W_comb = ((W_up @ W_end) * 64.0).astype(BF)
    bend_eff = f(inputs["b_end"]) + f(inputs["b_up"]) @ W_end

    def dr8(w, n_out):  # [512, n_out] -> [kt'=2, ki=128, ko=2, n_out] e4m3 x64
        w8 = (w * 64.0).astype(F8N)
        return np.ascontiguousarray(
            w8.reshape(2, 2, 128, n_out).transpose(0, 2, 1, 3))

    col = lambda b, n: np.ascontiguousarray(f(b).reshape(n, 128).T)
    shared = {
        "W_lin8": dr8(f(inputs["W_lin"]), 2 * DIM),
        "W_down": f(inputs["W_down"]).astype(BF).reshape(4, 128, HID),
        "WqT": np.ascontiguousarray(f(inputs["Wq"]).T).astype(BF).reshape(2, 128, HID),
        "WkT": np.ascontiguousarray(f(inputs["Wk"]).T).astype(BF).reshape(2, 128, HID),
        "WvT": np.ascontiguousarray(f(inputs["Wv"]).T).astype(BF).reshape(2, 128, HID),
        "WoT": np.ascontiguousarray(f(inputs["Wo"]).T).astype(BF).reshape(2, 128, HID),
        "W_end8": dr8(W_end, DIM),
        "W_comb": np.ascontiguousarray(W_comb).reshape(2, 128, DIM),
        "b_lin_pg": col(inputs["b_lin"], 8),
        "b_down_pg": col(inputs["b_down"], 2),
        "b_end_pg": np.ascontiguousarray(bend_eff.reshape(4, 128).T),
        "gamma_pg": col(inputs["gamma"], 4),
        "beta_pg": col(inputs["beta"], 4),
        "lntau_col": temp_col,
        "ones_in": np.ones((128, 128), BF),
        "ident_in": np.eye(128, dtype=np.float32).astype(BF),
        "id64_in": (64.0 * np.eye(128, dtype=np.float32)).astype(BF),
    }
    in_maps = []
    for c in range(8):
        m = dict(shared)
        m["x1"] = _reorder_x(x1[8 * c:8 * (c + 1)], BF)
        m["x1_8"] = _reorder_x(x1[8 * c:8 * (c + 1)], F8N)
        m["x2_8"] = _reorder_x(x2[8 * c:8 * (c + 1)], F8N)
        in_maps.append(m)
    return in_maps


def run_in_maps(in_maps):
    """Run the prebuilt executable on 8 cores; returns per-core out arrays."""
    import jax
    fn, in_names, out_names, out_avals, zero_outs = _get_runner()
    per_core = [[np.asarray(m[name]) for name in in_names] for m in in_maps]
    concat_in = [np.concatenate([per_core[c][i] for c in range(8)], axis=0)
                 for i in range(len(in_names))]
    concat_zeros = [np.zeros((8 * z.shape[0], *z.shape[1:]), z.dtype)
                    for z in zero_outs]
    out = fn(*concat_in, *concat_zeros)
    jax.block_until_ready(out)
    oi = out_names.index("out")
    arr = np.asarray(out[oi]).reshape(8, *out_avals[oi].shape)
    return arr


def kernel(**inputs):
    in_maps = _prep_inputs(inputs)
    arr = run_in_maps(in_maps)  # [8, 5, 4, 128, 512] bf16
    full = np.empty((64, DIM, NTOK), np.float32)
    for c in range(8):
        full[8 * c:8 * (c + 1)] = _unreorder_out(arr[c])
    return full.reshape(64, DIM, 16, 20).astype(np.float32)


if __name__ == "__main__":
    rng = np.random.default_rng(0)
    ins = {
        "x1": rng.standard_normal((64, 512, 16, 20), dtype=np.float32),
        "x2": rng.standard_normal((64, 512, 16, 20), dtype=np.float32),
    }
    s = 0.02
    for nm, shape in [("W_lin", (512, 1024)), ("W_down", (512, 256)),
                      ("W_up", (256, 512)), ("Wq", (256, 256)),
                      ("Wk", (256, 256)), ("Wv", (256, 256)),
                      ("Wo", (256, 256)), ("W_end", (512, 512))]:
        ins[nm] = (rng.standard_normal(shape) * s).astype(np.float32)
    for nm, n in [("b_lin", 1024), ("b_down", 256), ("b_up", 512),
                  ("b_end", 512)]:
        ins[nm] = np.zeros(n, np.float32)
    ins["gamma"] = np.ones(512, np.float32)
    ins["beta"] = np.zeros(512, np.float32)
    ins["temperature"] = np.ones((8, 1, 1), np.float32)
    out = kernel(**ins)
    print("kernel ran, out shape", out.shape, "mean", float(np.abs(out).mean()))


# revision 15
# speedup vs baseline: 1.0761x; 1.0761x over previous
"""Trainium2 Bass kernel for nn_Attention_Module (dense_transformer).

Data-parallel over batch: B=64 split across 8 NeuronCores (8 per core).
Per core, activations are channel-major [C, tokens] with the 8 local
batches' 320 tokens reordered host-side into a z-block tile (8*64=512
template tokens) + 4 x-block tiles (2 batches x 256 search tokens each).

v3: bf16 activations/weights (PSUM accumulation fp32); single scalar
activation-table set (ln+exp; rsqrt = exp(-0.5 ln x)); transposed
attention scores G^T = kT.T @ qT with the k-side row norm folded into
the Exp's per-partition scale and one full 128x128 Exp per
(branch, group); softmax denominator via a ones column interleaved
into v (one matmul emits AV and the row sum); per-head 32x32
tile_position-packed AV; residuals accumulated into PSUM via identity
matmuls; W_up@W_end folded host-side into W_comb; fully contiguous
host-reordered DMA; interleaved front/back emission for PE density.

Self-contained: only imports infra from /opt/trn_rl_repo.
"""
import sys

sys.path.insert(0, "/opt/trn_rl_repo")

from contextlib import ExitStack

import numpy as np

import bass_rust as _bass_rust
import concourse.bacc as bacc
import concourse.tile as tile
from concourse import mybir
from concourse.hw_specs import get_activation_tables

F32 = mybir.dt.float32
BF16 = mybir.dt.bfloat16
F8 = mybir.dt.float8e4
SC = 64.0  # fp8 weight pre-scale (folded back out via activation scales)
AF = mybir.ActivationFunctionType
OP = mybir.AluOpType
AX = mybir.AxisListType

B_LOC = 8          # batches per core
DIM = 512
HID = 256
HEADS = 8
NZ, NX = 64, 256   # template / search tokens per batch
NTOK = NZ + NX     # 320
NT = 5             # token tiles of 512
EPS_LN = 1e-5
TINY = 1e-24       # guards ln of exact-zero row norms
VEXT = 528         # per-group v_ext row: max(8*65, 2*257) padded


def _bbs(j):
    """Branch segments inside token-tile j: list of (col_off, width)."""
    if j == 0:
        return [(64 * b, 64) for b in range(B_LOC)]
    return [(0, 256), (256, 256)]


def _chunks(off, w):
    """Token-partition chunks (tb, part_off, part_w) for a branch segment."""
    if w == 64:
        return [(off // 128, off % 128, 64)]
    return [(off // 128, 0, 128), (off // 128 + 1, 0, 128)]


def build_nc():
    nc = bacc.Bacc("TRN2", target_bir_lowering=False, debug=False,
                   num_devices=8)

    # ---- DRAM I/O (per-core shapes, host pre-reordered / pre-cast) ----
    x1_e = nc.declare_dram_parameter("x1", [NT, 4, 128, 512], BF16, isOutput=False)
    x18_e = nc.declare_dram_parameter("x1_8", [NT, 4, 128, 512], F8, isOutput=False)
    x28_e = nc.declare_dram_parameter("x2_8", [NT, 4, 128, 512], F8, isOutput=False)
    out_e = nc.declare_dram_parameter("out", [NT, 4, 128, 512], BF16, isOutput=True)
    wlin8_e = nc.declare_dram_parameter("W_lin8", [2, 128, 2, 2 * DIM], F8, isOutput=False)
    wdown_e = nc.declare_dram_parameter("W_down", [4, 128, HID], BF16, isOutput=False)
    wq_e = nc.declare_dram_parameter("WqT", [2, 128, HID], BF16, isOutput=False)
    wk_e = nc.declare_dram_parameter("WkT", [2, 128, HID], BF16, isOutput=False)
    wv_e = nc.declare_dram_parameter("WvT", [2, 128, HID], BF16, isOutput=False)
    wo_e = nc.declare_dram_parameter("WoT", [2, 128, HID], BF16, isOutput=False)
    wend8_e = nc.declare_dram_parameter("W_end8", [2, 128, 2, DIM], F8, isOutput=False)
    wcomb_e = nc.declare_dram_parameter("W_comb", [2, 128, DIM], BF16, isOutput=False)
    blin_e = nc.declare_dram_parameter("b_lin_pg", [128, 8], F32, isOutput=False)
    bdown_e = nc.declare_dram_parameter("b_down_pg", [128, 2], F32, isOutput=False)
    bend_e = nc.declare_dram_parameter("b_end_pg", [128, 4], F32, isOutput=False)
    gamma_e = nc.declare_dram_parameter("gamma_pg", [128, 4], F32, isOutput=False)
    beta_e = nc.declare_dram_parameter("beta_pg", [128, 4], F32, isOutput=False)
    temp_e = nc.declare_dram_parameter("lntau_col", [128, 2], F32, isOutput=False)
    ones_e = nc.declare_dram_parameter("ones_in", [128, 128], BF16, isOutput=False)
    ident_e = nc.declare_dram_parameter("ident_in", [128, 128], BF16, isOutput=False)
    id64_e = nc.declare_dram_parameter("id64_in", [128, 128], BF16, isOutput=False)

    with tile.TileContext(nc) as tc, ExitStack() as ctx:
        wts = ctx.enter_context(tc.tile_pool(name="wts", bufs=1))
        xload = ctx.enter_context(tc.tile_pool(name="xload", bufs=3))
        u1p = ctx.enter_context(tc.tile_pool(name="u1p", bufs=1))
        rp = ctx.enter_context(tc.tile_pool(name="rp", bufs=2))
        u2p = ctx.enter_context(tc.tile_pool(name="u2p", bufs=1))
        ap_ = ctx.enter_context(tc.tile_pool(name="ap", bufs=2))
        bqp = ctx.enter_context(tc.tile_pool(name="bqp", bufs=1))
        qkvp = ctx.enter_context(tc.tile_pool(name="qkvp", bufs=2))
        sqp = ctx.enter_context(tc.tile_pool(name="sqp", bufs=2))
        nrmp = ctx.enter_context(tc.tile_pool(name="nrmp", bufs=2))
        qtp = ctx.enter_context(tc.tile_pool(name="qtp", bufs=2))
        etp = ctx.enter_context(tc.tile_pool(name="etp", bufs=3))
        rp2 = ctx.enter_context(tc.tile_pool(name="rp2", bufs=2))
        avp = ctx.enter_context(tc.tile_pool(name="avp", bufs=1))
        o1p = ctx.enter_context(tc.tile_pool(name="o1p", bufs=1))
        scr = ctx.enter_context(tc.tile_pool(name="scr", bufs=2))
        prep = ctx.enter_context(tc.tile_pool(name="prep", bufs=1))
        statp = ctx.enter_context(tc.tile_pool(name="statp", bufs=1))
        outp = ctx.enter_context(tc.tile_pool(name="outp", bufs=1))
        ps = ctx.enter_context(tc.tile_pool(name="ps", bufs=2, space="PSUM"))
        pst = ctx.enter_context(tc.tile_pool(name="pst", bufs=2, space="PSUM"))
        psg = ctx.enter_context(tc.tile_pool(name="psg", bufs=2, space="PSUM"))
        psav = ctx.enter_context(tc.tile_pool(name="psav", bufs=2, space="PSUM"))

        # ---- weights / constants in SBUF ----
        wlin8_sb = wts.tile([128, 2, 2, 2 * DIM], F8)
        wdown_sb = wts.tile([128, 4, HID], BF16)
        wq_sb = wts.tile([128, 2, HID], BF16)
        wk_sb = wts.tile([128, 2, HID], BF16)
        wv_sb = wts.tile([128, 2, HID], BF16)
        wo_sb = wts.tile([128, 2, HID], BF16)
        wend8_sb = wts.tile([128, 2, 2, DIM], F8)
        wcomb_sb = wts.tile([128, 2, DIM], BF16)

        blin_sb = wts.tile([128, 8], F32)
        bdown_sb = wts.tile([128, 2], F32)
        bend_sb = wts.tile([128, 4], F32)
        gamma_sb = wts.tile([128, 4], F32)
        beta_sb = wts.tile([128, 4], F32)
        tempc_sb = wts.tile([128, 2], F32)
        ones_sb = wts.tile([128, 128], BF16)
        ident_sb = wts.tile([128, 128], BF16)
        id64_sb = wts.tile([128, 128], BF16)
        tiny_sb = wts.tile([128, 1], F32)
        nc.vector.memset(tiny_sb[:], TINY)
        epsln_sb = wts.tile([128, 1], F32)
        nc.vector.memset(epsln_sb[:], EPS_LN)
        zero_sb = wts.tile([128, 1], F32)
        nc.vector.memset(zero_sb[:], 0.0)

        def emit_loads(j):
            x1t8 = xload.tile([128, 4, 512], F8, tag="x1l8")
            x2t8 = xload.tile([128, 4, 512], F8, tag="x2l8")
            x1t = xload.tile([128, 4, 512], BF16, tag="x1l")
            nc.sync.dma_start(x1t8[:], x18_e[j].rearrange("k p t -> p k t"))
            nc.sync.dma_start(x2t8[:], x28_e[j].rearrange("k p t -> p k t"))
            nc.sync.dma_start(x1t[:], x1_e[j].rearrange("k p t -> p k t"))
            return (x1t, x1t8, x2t8)

        def emit_front_a(j, ld):
            x1t, x1t8, x2t8 = ld
            # ---- S1: h1 = relu((W_lin8^T X1_8)/SC + b); r = y1 + u1 (fp8) ----
            u1 = u1p.tile([128, 4, 512], BF16)
            r = rp.tile([128, 4, 512], F8)
            for m in [4, 5, 6, 7, 0, 1, 2, 3]:
                pt = ps.tile([128, 512], F32, tag="ps")
                for kt in range(2):
                    nc.tensor.matmul(pt[:], wlin8_sb[:, kt, :, 128 * m:128 * (m + 1)],
                                     x1t8[:, 2 * kt:2 * kt + 2, :],
                                     start=(kt == 0), stop=(kt == 1),
                                     perf_mode=mybir.MatmulPerfMode.DoubleRow)
                if m >= 4:
                    nc.scalar.activation(u1[:, m - 4, :], pt[:], AF.Relu,
                                         bias=blin_sb[:, m:m + 1], scale=1.0 / SC)
                else:
                    ytmp = scr.tile([128, 512], BF16, tag="ytmp")
                    nc.scalar.activation(ytmp[:], pt[:], AF.Relu,
                                         bias=blin_sb[:, m:m + 1], scale=1.0 / SC)
                    nc.gpsimd.tensor_add(r[:, m, :], ytmp[:], u1[:, m, :])
            return dict(x1t=x1t, x2t8=x2t8, u1=u1, r=r)

        def emit_front_s1b(j, st):
            x2t8 = st["x2t8"]
            # ---- S1b: u2 = relu(W_lin[:,512:]^T X2 + b2) ----
            u2 = u2p.tile([128, 4, 512], BF16)
            for m in range(4):
                pt = ps.tile([128, 512], F32, tag="ps")
                for kt in range(2):
                    nc.tensor.matmul(
                        pt[:],
                        wlin8_sb[:, kt, :, 512 + 128 * m:512 + 128 * (m + 1)],
                        x2t8[:, 2 * kt:2 * kt + 2, :],
                        start=(kt == 0), stop=(kt == 1),
                        perf_mode=mybir.MatmulPerfMode.DoubleRow)
                nc.scalar.activation(u2[:, m, :], pt[:], AF.Relu,
                                     bias=blin_sb[:, 4 + m:5 + m], scale=1.0 / SC)
            st["u2"] = u2

        def emit_front_b(j, st):
            u1, u2 = st["u1"], st["u2"]
            bbs = _bbs(j)
            nb = len(bbs)
            w_ = bbs[0][1]

            # ---- S2: A = relu(W_down^T u1 + b_down); Bq likewise from u2 ----
            A = ap_.tile([128, 2, 512], BF16)
            Bq = bqp.tile([128, 2, 512], BF16)
            for (dst, src) in ((A, u1), (Bq, u2)):
                for m in range(2):
                    pt = ps.tile([128, 512], F32, tag="ps")
                    for kt in range(4):
                        nc.tensor.matmul(pt[:],
                                         wdown_sb[:, kt, 128 * m:128 * (m + 1)],
                                         src[:, kt, :],
                                         start=(kt == 0), stop=(kt == 3))
                    nc.scalar.activation(dst[:, m, :], pt[:], AF.Relu,
                                         bias=bdown_sb[:, m:m + 1])

            # ---- S3: q = Wq@Bq, k = Wk@A (channel-major, SBUF copies);
            #          v -> v_ext with a ones column per branch segment ----
            q = qkvp.tile([128, 2, 512], BF16, tag="q")
            k = qkvp.tile([128, 2, 512], BF16, tag="k")
            vx = qkvp.tile([128, 2, VEXT], BF16, tag="vx")
            for (dst, w_sb, src) in ((q, wq_sb, Bq), (k, wk_sb, A)):
                for m in range(2):
                    pt = ps.tile([128, 512], F32, tag="ps")
                    for kt in range(2):
                        nc.tensor.matmul(pt[:], w_sb[:, kt, 128 * m:128 * (m + 1)],
                                         src[:, kt, :],
                                         start=(kt == 0), stop=(kt == 1))
                    nc.vector.tensor_copy(dst[:, m, :], pt[:])
            for m in range(2):
                pt = ps.tile([128, 512], F32, tag="ps")
                for kt in range(2):
                    nc.tensor.matmul(pt[:], wv_sb[:, kt, 128 * m:128 * (m + 1)],
                                     A[:, kt, :], start=(kt == 0), stop=(kt == 1))
                vxg = vx[:, m, 0:nb * (w_ + 1)].rearrange(
                    "p (n e) -> p n e", e=w_ + 1)
                nc.vector.memset(vxg[:, :, w_:w_ + 1], 1.0)
                nc.vector.tensor_copy(
                    vxg[:, :, 0:w_],
                    pt[:].rearrange("p (n w) -> p n w", w=w_))
            # kT transpose needs no norm scaling: do it as soon as k lands
            kT = qtp.tile([128, 4, 256], BF16, tag="kT")
            for tb in range(4):
                pt = pst.tile([128, 256], BF16, tag="pst")
                for g in range(2):
                    nc.tensor.matmul(
                        pt[:, 128 * g:128 * (g + 1)],
                        k[:, g, 128 * tb:128 * (tb + 1)], ident_sb[:],
                        is_transpose=True, start=(g == 0), stop=(g == 1))
                nc.vector.tensor_copy(kT[:, tb, :], pt[:])
            return dict(A=A, q=q, k=k, vx=vx, kT=kT)

        def emit_back_a(j, st):
            bbs = _bbs(j)
            nb = len(bbs)
            q, k = st["q"], st["k"]
            # ---- S4: rsqrt(row L2 norms) = exp(-0.5 ln(ssq)) ----
            w = 512 // nb
            rn = {}
            for (name, t_) in (("q", q), ("k", k)):
                sq = sqp.tile([128, 2, 512], BF16, tag="sq")
                nc.vector.tensor_mul(sq[:], t_[:], t_[:])
                ssq = nrmp.tile([128, 2, nb], F32, tag="ssq" + name)
                nc.vector.reduce_sum(
                    ssq[:], sq[:].rearrange("p g (n w) -> p g n w", w=w), axis=AX.X)
                lnt = nrmp.tile([128, 2, nb], F32, tag="ln" + name)
                nc.scalar.activation(lnt[:], ssq[:], AF.Ln, bias=tiny_sb[:, 0:1])
                rr = nrmp.tile([128, 2, nb], F32, tag="rn" + name)
                if name == "q":  # fold per-head temperature: exp(-ln(ssq)/2 + ln tau)
                    for g in range(2):
                        nc.scalar.activation(rr[:, g, :], lnt[:, g, :], AF.Exp,
                                             scale=-0.5,
                                             bias=tempc_sb[:, g:g + 1])
                else:
                    nc.scalar.activation(rr[:], lnt[:], AF.Exp, scale=-0.5)
                rn[name] = rr
            for g in range(2):
                for bi, (off, w_) in enumerate(bbs):
                    nc.vector.tensor_scalar_mul(
                        q[:, g, off:off + w_], in0=q[:, g, off:off + w_],
                        scalar1=rn["q"][:, g, bi:bi + 1])
            st["rnk"] = rn["k"]

        def emit_back_attn(j, st):
            bbs = _bbs(j)
            nb = len(bbs)
            q, vx, rnk, kT = st["q"], st["vx"], st["rnk"], st["kT"]
            # ---- S5: PE-transpose scaled q -> token-major qT ----
            qT = qtp.tile([128, 4, 256], BF16, tag="qT")
            for tb in range(4):
                pt = pst.tile([128, 256], BF16, tag="pst")
                for g in range(2):
                    nc.tensor.matmul(
                        pt[:, 128 * g:128 * (g + 1)],
                        q[:, g, 128 * tb:128 * (tb + 1)], ident_sb[:],
                        is_transpose=True, start=(g == 0), stop=(g == 1))
                nc.vector.tensor_copy(qT[:, tb, :], pt[:])

            # ---- S6-S8: per (branch, group): G^T -> exp(scale=rn_k) ->
            #      AV matmul with interleaved ones col -> R=1/S -> scale ----
            av = avp.tile([128, 2, 512], BF16)
            for bi, (off, w_) in enumerate(bbs):
                chunks = _chunks(off, w_)
                for g in range(2):
                    gps = psg.tile([128, 128], F32, tag="gps")
                    for ci, (tb, tpo, cw) in enumerate(chunks):
                        nc.tensor.matmul(
                            gps[:],
                            kT[tpo:tpo + cw, tb, 128 * g:128 * (g + 1)],
                            qT[tpo:tpo + cw, tb, 128 * g:128 * (g + 1)],
                            start=(ci == 0), stop=(ci == len(chunks) - 1))
                    ET = etp.tile([128, 128], BF16, tag="et")
                    nc.scalar.activation(ET[:], gps[:], AF.Exp,
                                         bias=zero_sb[:, 0:1],
                                         scale=rnk[:, g, bi:bi + 1])
                    pav = psav.tile([128, 512], F32, tag="pav")
                    e1 = w_ + 1
                    for h in range(4):
                        hs = slice(32 * h, 32 * (h + 1))
                        nc.tensor.matmul(
                            pav[hs, 0:e1], ET[hs, hs],
                            vx[hs, g, bi * e1:(bi + 1) * e1],
                            start=True, stop=True, tile_position=(32 * h, 32 * h))
                    R = rp2.tile([128, 1], F32, tag="R")
                    nc.vector.reciprocal_approx_fast(R[:], pav[:, w_:w_ + 1])
                    nc.scalar.mul(av[:, g, off:off + w_], pav[:, 0:w_],
                                  mul=R[:, 0:1])

            st["av"] = av

        def emit_back_tail(j, st):
            x1t, r, A, av = st["x1t"], st["r"], st["A"], st["av"]
            # ---- S9: o1 = Wo@av + A (A added via identity matmul) ----
            o1 = o1p.tile([128, 2, 512], BF16)
            for m in range(2):
                pt = ps.tile([128, 512], F32, tag="ps")
                for kt in range(2):
                    nc.tensor.matmul(pt[:], wo_sb[:, kt, 128 * m:128 * (m + 1)],
                                     av[:, kt, :], start=(kt == 0), stop=False)
                nc.tensor.matmul(pt[:], ident_sb[:], A[:, m, :],
                                 start=False, stop=True)
                nc.vector.tensor_copy(o1[:, m, :], pt[:])

            # ---- S10/S11: pre = W_end^T r + W_comb^T o1 + t1 + b_eff ----
            pre = prep.tile([128, 4, 512], BF16)
            s1ps = psav.tile([128, 512], F32, tag="pav")
            s2ps = psav.tile([128, 512], F32, tag="pav")
            for m in range(4):
                pt = ps.tile([128, 512], F32, tag="ps")
                for kt in range(2):
                    nc.tensor.matmul(pt[:], wend8_sb[:, kt, :, 128 * m:128 * (m + 1)],
                                     r[:, 2 * kt:2 * kt + 2, :],
                                     start=(kt == 0), stop=False,
                                     perf_mode=mybir.MatmulPerfMode.DoubleRow)
                for kt in range(2):
                    nc.tensor.matmul(pt[:], wcomb_sb[:, kt, 128 * m:128 * (m + 1)],
                                     o1[:, kt, :], start=False, stop=False)
                nc.tensor.matmul(pt[:], id64_sb[:], x1t[:, m, :],
                                 start=False, stop=True)
                nc.vector.tensor_scalar(pre[:, m, :], in0=pt[:],
                                        scalar1=1.0 / SC,
                                        scalar2=bend_sb[:, m:m + 1],
                                        op0=OP.mult, op1=OP.add)
                p2 = scr.tile([128, 512], BF16, tag="p2")
                nc.scalar.activation(p2[:], pre[:, m, :], AF.Square)
                nc.tensor.matmul(s1ps[:], ones_sb[:], pre[:, m, :],
                                 start=(m == 0), stop=(m == 3))
                nc.tensor.matmul(s2ps[:], ones_sb[:], p2[:],
                                 start=(m == 0), stop=(m == 3))

            # ---- S12: mu/rstd (rows replicated); rstd = exp(-0.5 ln(var)) ----
            mu = statp.tile([128, 512], BF16, tag="mu")
            nc.vector.tensor_scalar_mul(mu[:], in0=s1ps[:], scalar1=1.0 / DIM)
            msq = statp.tile([128, 512], BF16, tag="msq")
            nc.vector.tensor_mul(msq[:], mu[:], mu[:])
            var = statp.tile([128, 512], F32, tag="var")
            nc.vector.scalar_tensor_tensor(var[:], in0=s2ps[:], scalar=1.0 / DIM,
                                           in1=msq[:], op0=OP.mult, op1=OP.subtract)
            lnv = statp.tile([128, 512], F32, tag="lnv")
            nc.scalar.activation(lnv[:], var[:], AF.Ln, bias=epsln_sb[:, 0:1])
            rstd = statp.tile([128, 512], BF16, tag="rstd")
            nc.scalar.activation(rstd[:], lnv[:], AF.Exp, scale=-0.5)

            # ---- S13: out = ((pre - mu) * rstd) * gamma + beta ----
            ot = outp.tile([128, 4, 512], BF16)
            for m in range(4):
                t1 = scr.tile([128, 512], BF16, tag="t1")
                nc.gpsimd.tensor_sub(t1[:], pre[:, m, :], mu[:])
                mgb = scr.tile([128, 512], BF16, tag="mgb")
                nc.vector.tensor_mul(mgb[:], t1[:], rstd[:])
                nc.vector.tensor_scalar(
                    ot[:, m, :], in0=mgb[:], scalar1=gamma_sb[:, m:m + 1],
                    scalar2=beta_sb[:, m:m + 1], op0=OP.mult, op1=OP.add)

            # ---- S14: store ----
            nc.sync.dma_start(out_e[j].rearrange("k p t -> p k t"), ot[:])

        order = [1, 2, 0, 3, 4]
        nc.scalar.dma_start(wlin8_sb[:, 0], wlin8_e[0].rearrange("p o m -> p o m"))
        nc.scalar.dma_start(blin_sb[:], blin_e[:, :])
        nc.gpsimd.dma_start(wlin8_sb[:, 1], wlin8_e[1].rearrange("p o m -> p o m"))
        ld = emit_loads(order[0])
        nc.gpsimd.dma_start(wdown_sb[:], wdown_e.rearrange("k p m -> p k m"))
        nc.sync.dma_start(wq_sb[:], wq_e.rearrange("k p m -> p k m"))
        nc.sync.dma_start(wk_sb[:], wk_e.rearrange("k p m -> p k m"))
        nc.sync.dma_start(wv_sb[:], wv_e.rearrange("k p m -> p k m"))
        nc.sync.dma_start(wo_sb[:], wo_e.rearrange("k p m -> p k m"))
        nc.sync.dma_start(wend8_sb[:], wend8_e.rearrange("k p o m -> p k o m"))
        nc.sync.dma_start(wcomb_sb[:], wcomb_e.rearrange("k p m -> p k m"))
        nc.sync.dma_start(bdown_sb[:], bdown_e[:, :])
        nc.sync.dma_start(bend_sb[:], bend_e[:, :])
        nc.sync.dma_start(gamma_sb[:], gamma_e[:, :])
        nc.sync.dma_start(beta_sb[:], beta_e[:, :])
        nc.sync.dma_start(tempc_sb[:], temp_e[:, :])
        nc.sync.dma_start(ones_sb[:], ones_e[:, :])
        nc.sync.dma_start(ident_sb[:], ident_e[:, :])
        nc.sync.dma_start(id64_sb[:], id64_e[:, :])

        st = emit_front_a(order[0], ld)
        emit_front_s1b(order[0], st)
        st.update(emit_front_b(order[0], st))
        prev = (order[0], st)
        for j in order[1:]:
            ld = emit_loads(j)
            emit_back_a(prev[0], prev[1])
            st = emit_front_a(j, ld)
            emit_front_s1b(j, st)
            emit_back_attn(prev[0], prev[1])
            st.update(emit_front_b(j, st))
            emit_back_tail(prev[0], prev[1])
            prev = (j, st)
        emit_back_a(prev[0], prev[1])
        emit_back_attn(prev[0], prev[1])
        emit_back_tail(prev[0], prev[1])

    # Run the act-table insertion pass with a curated set list so every
    # activation (relu/ln/exp/square/copy) resolves to the one combined
    # natural_log_exp_and_others set -> a single ACT_TABLE_LOAD.
    shared = {AF.Exp, AF.Ln, AF.Relu, AF.Square, AF.Copy, AF.Identity}
    tabs = get_activation_tables(nc.m.arch)
    curated = []
    for name, fns in tabs.items():
        if name != "natural_log_exp_and_others":
            fns = fns - shared
        curated.append((name, fns))

    orig = bacc.Bacc.insert_act_table_loads

    def _curated(self):
        _bass_rust.insert_act_table_loads(self, curated)

    nc.insert_act_table_loads = _curated.__get__(nc)
    try:
        nc.compile()
    finally:
        nc.insert_act_table_loads = orig.__get__(nc)
    return nc


# ---------------- host side ----------------
_CACHE = {}


def _get_runner():
    if "runner" in _CACHE:
        return _CACHE["runner"]
    import jax
    from jax.sharding import Mesh, PartitionSpec
    from jax.experimental.shard_map import shard_map
    from concourse.bass2jax import (
        _bass_exec_p, install_neuronx_cc_hook, partition_id_tensor)
    import concourse.mybir as mybir_

    nc = build_nc()
    install_neuronx_cc_hook()
    partition_name = nc.partition_id_tensor.name if nc.partition_id_tensor else None
    in_names, out_names, out_avals, zero_outs = [], [], [], []
    for alloc in nc.m.functions[0].allocations:
        if not isinstance(alloc, mybir_.MemoryLocationSet):
            continue
        name = alloc.memorylocations[0].name
        if alloc.kind == "ExternalInput":
            if name != partition_name:
                in_names.append(name)
        elif alloc.kind == "ExternalOutput":
            out_names.append(name)
            shape = tuple(alloc.tensor_shape)
            dtype = mybir_.dt.np(alloc.dtype)
            out_avals.append(jax.core.ShapedArray(shape, dtype))
            zero_outs.append(np.zeros(shape, dtype))
    n_params, n_outs = len(in_names), len(out_avals)
    all_in = list(in_names) + list(out_names)
    if partition_name is not None:
        all_in.append(partition_name)
    donate = tuple(range(n_params, n_params + n_outs))

    def _body(*args):
        operands = list(args)
        if partition_name is not None:
            operands.append(partition_id_tensor())
        return tuple(_bass_exec_p.bind(
            *operands, out_avals=tuple(out_avals), in_names=tuple(all_in),
            out_names=tuple(out_names), lowering_input_output_aliases=(),
            sim_require_finite=True, sim_require_nnan=True, nc=nc))

    devices = jax.devices()[:8]
    mesh = Mesh(np.asarray(devices), ("core",))
    fn = jax.jit(
        shard_map(_body, mesh=mesh,
                  in_specs=(PartitionSpec("core"),) * (n_params + n_outs),
                  out_specs=(PartitionSpec("core"),) * n_outs,
                  check_rep=False),
        donate_argnums=donate, keep_unused=True)
    _CACHE["runner"] = (fn, in_names, out_names, out_avals, zero_outs)
    return _CACHE["runner"]


def _reorder_x(xc, BF):
    """[8, 512, 320] fp32 -> [5, 4, 128, 512] bf16, token-reordered."""
    dev = np.empty((NT, 4, 128, 512), dtype=BF)
    z = np.transpose(xc[:, :, 0:64], (1, 0, 2)).reshape(512, 512)
    dev[0] = z.reshape(4, 128, 512).astype(BF)
    for j in range(1, NT):
        xx = np.transpose(xc[2 * j - 2:2 * j, :, 64:320], (1, 0, 2))
        dev[j] = xx.reshape(512, 512).reshape(4, 128, 512).astype(BF)
    return dev


def _unreorder_out(dev):
    """[5, 4, 128, 512] (any float) -> [8, 512, 320] fp32."""
    out = np.empty((B_LOC, DIM, NTOK), np.float32)
    z = dev[0].astype(np.float32).reshape(512, 8, 64)
    out[:, :, 0:64] = np.transpose(z, (1, 0, 2))
    for j in range(1, NT):
        xx = dev[j].astype(np.float32).reshape(512, 2, 256)
        out[2 * j - 2:2 * j, :, 64:320] = np.transpose(xx, (1, 0, 2))
    return out


def _prep_inputs(inputs):
    import ml_dtypes
    BF = ml_dtypes.bfloat16
    f = lambda a: np.ascontiguousarray(np.asarray(a), dtype=np.float32)
    x1 = f(inputs["x1"]).reshape(64, DIM, NTOK)
    x2 = f(inputs["x2"]).reshape(64, DIM, NTOK)
    temp = np.log(f(inputs["temperature"]).reshape(HEADS))
    temp_col = np.empty((128, 2), np.float32)
    for g in range(2):
        for hh in range(4):
            temp_col[32 * hh:32 * (hh + 1), g] = temp[4 * g + hh]

    F8N = ml_dtypes.float8_e4m3
    W_up = f(inputs["W_up"])
    W_end = f(inputs["W_end"])
    W_comb = ((W_up @ W_end) * 64.0).astype(BF)
    bend_eff = f(inputs["b_end"]) + f(inputs["b_up"]) @ W_end

    def dr8(w, n_out):  # [512, n_out] -> [kt'=2, ki=128, ko=2, n_out] e4m3 x64
        w8 = (w * 64.0).astype(F8N)
        return np.ascontiguousarray(
            w8.reshape(2, 2, 128, n_out).transpose(0, 2, 1, 3))

    col = lambda b, n: np.ascontiguousarray(f(b).reshape(n, 128).T)
    shared = {
        "W_lin8": dr8(f(inputs["W_lin"]), 2 * DIM),
        "W_down": f(inputs["W_down"]).astype(BF).reshape(4, 128, HID),
        "WqT": np.ascontiguousarray(f(inputs["Wq"]).T).astype(BF).reshape(2, 128, HID),
        "WkT": np.ascontiguousarray(f(inputs["Wk"]).T).astype(BF).reshape(2, 128, HID),
        "WvT": np.ascontiguousarray(f(inputs["Wv"]).T).astype(BF).reshape(2, 128, HID),
        "WoT": np.ascontiguousarray(f(inputs["Wo"]).T).astype(BF).reshape(2, 128, HID),
        "W_end8": dr8(W_end, DIM),
        "W_comb": np.ascontiguousarray(W_comb).reshape(2, 128, DIM),
        "b_lin_pg": col(inputs["b_lin"], 8),
        "b_down_pg": col(inputs["b_down"], 2),
        "b_end_pg": np.ascontiguousarray(bend_eff.reshape(4, 128).T),
        "gamma_pg": col(inputs["gamma"], 4),
        "beta_pg": col(inputs["beta"], 4),
        "lntau_col": temp_col,
        "ones_in": np.ones((128, 128), BF),
        "ident_in": np.eye(128, dtype=np.float32).astype(BF),
        "id64_in": (64.0 * np.eye(128, dtype=np.float32)).astype(BF),
    }
    in_maps = []
    for c in range(8):
        m = dict(shared)
        m["x1"] = _reorder_x(x1[8 * c:8 * (c + 1)], BF)
        m["x1_8"] = _reorder_x(x1[8 * c:8 * (c + 1)], F8N)
        m["x2_8"] = _reorder_x(x2[8 * c:8 * (c + 1)], F8N)
        in_maps.append(m)
    return in_maps


def run_in_maps(in_maps):
    """Run the prebuilt executable on 8 cores; returns per-core out arrays."""
    import jax
    fn, in_names, out_names, out_avals, zero_outs = _get_runner()
    per_core = [[np.asarray(m[name]) for name in in_names] for m in in_maps]
    concat_in = [np.concatenate([per_core[c][i] for c in range(8)], axis=0)
                 for i in range(len(in_names))]
    concat_zeros = [np.zeros((8 * z.shape[0], *z.shape[1:]), z.dtype)
                    for z in zero_outs]
    out = fn(*concat_in, *concat_zeros)
    jax.block_until_ready(out)
    oi = out_names.index("out")
    arr = np.asarray(out[oi]).reshape(8, *out_avals[oi].shape)
    return arr


def kernel(**inputs):
    in_maps = _prep_inputs(inputs)
    arr = run_in_maps(in_maps)  # [8, 5, 4, 128, 512] bf16
    full = np.empty((64, DIM, NTOK), np.float32)
    for c in range(8):
        full[8 * c:8 * (c + 1)] = _unreorder_out(arr[c])
    return full.reshape(64, DIM, 16, 20).astype(np.float32)


if __name__ == "__main__":
    rng = np.random.default_rng(0)
    ins = {
        "x1": rng.standard_normal((64, 512, 16, 20), dtype=np.float32),
        "x2": rng.standard_normal((64, 512, 16, 20), dtype=np.float32),
    }
    s = 0.02
    for nm, shape in [("W_lin", (512, 1024)), ("W_down", (512, 256)),
                      ("W_up", (256, 512)), ("Wq", (256, 256)),
                      ("Wk", (256, 256)), ("Wv", (256, 256)),
                      ("Wo", (256, 256)), ("W_end", (512, 512))]:
        ins[nm] = (rng.standard_normal(shape) * s).astype(np.float32)
    for nm, n in [("b_lin", 1024), ("b_down", 256), ("b_up", 512),
                  ("b_end", 512)]:
        ins[nm] = np.zeros(n, np.float32)
    ins["gamma"] = np.ones(512, np.float32)
    ins["beta"] = np.zeros(512, np.float32)
    ins["temperature"] = np.ones((8, 1, 1), np.float32)
    out = kernel(**ins)
    print("kernel ran, out shape", out.shape, "mean", float(np.abs(out).mean()))


# revision 17
# speedup vs baseline: 1.0880x; 1.0110x over previous
"""Trainium2 Bass kernel for nn_Attention_Module (dense_transformer).

Data-parallel over batch: B=64 split across 8 NeuronCores (8 per core).
Per core, activations are channel-major [C, tokens] with the 8 local
batches' 320 tokens reordered host-side into a z-block tile (8*64=512
template tokens) + 4 x-block tiles (2 batches x 256 search tokens each).

v3: bf16 activations/weights (PSUM accumulation fp32); single scalar
activation-table set (ln+exp; rsqrt = exp(-0.5 ln x)); transposed
attention scores G^T = kT.T @ qT with the k-side row norm folded into
the Exp's per-partition scale and one full 128x128 Exp per
(branch, group); softmax denominator via a ones column interleaved
into v (one matmul emits AV and the row sum); per-head 32x32
tile_position-packed AV; residuals accumulated into PSUM via identity
matmuls; W_up@W_end folded host-side into W_comb; fully contiguous
host-reordered DMA; interleaved front/back emission for PE density.

Self-contained: only imports infra from /opt/trn_rl_repo.
"""
import sys

sys.path.insert(0, "/opt/trn_rl_repo")

from contextlib import ExitStack

import numpy as np

import bass_rust as _bass_rust
import concourse.bacc as bacc
import concourse.tile as tile
from concourse import mybir
from concourse.hw_specs import get_activation_tables

F32 = mybir.dt.float32
BF16 = mybir.dt.bfloat16
F8 = mybir.dt.float8e4
SC = 64.0  # fp8 weight pre-scale (folded back out via activation scales)
AF = mybir.ActivationFunctionType
OP = mybir.AluOpType
AX = mybir.AxisListType

B_LOC = 8          # batches per core
DIM = 512
HID = 256
HEADS = 8
NZ, NX = 64, 256   # template / search tokens per batch
NTOK = NZ + NX     # 320
NT = 5             # token tiles of 512
EPS_LN = 1e-5
TINY = 1e-24       # guards ln of exact-zero row norms
VEXT = 528         # per-group v_ext row: max(8*65, 2*257) padded


def _bbs(j):
    """Branch segments inside token-tile j: list of (col_off, width)."""
    if j == 0:
        return [(64 * b, 64) for b in range(B_LOC)]
    return [(0, 256), (256, 256)]


def _chunks(off, w):
    """Token-partition chunks (tb, part_off, part_w) for a branch segment."""
    if w == 64:
        return [(off // 128, off % 128, 64)]
    return [(off // 128, 0, 128), (off // 128 + 1, 0, 128)]


def build_nc():
    nc = bacc.Bacc("TRN2", target_bir_lowering=False, debug=False,
                   num_devices=8)

    # ---- DRAM I/O (per-core shapes, host pre-reordered / pre-cast) ----
    x1_e = nc.declare_dram_parameter("x1", [NT, 4, 128, 512], BF16, isOutput=False)
    x18_e = nc.declare_dram_parameter("x1_8", [NT, 4, 128, 512], F8, isOutput=False)
    x28_e = nc.declare_dram_parameter("x2_8", [NT, 4, 128, 512], F8, isOutput=False)
    out_e = nc.declare_dram_parameter("out", [NT, 4, 128, 512], BF16, isOutput=True)
    wlin8_e = nc.declare_dram_parameter("W_lin8", [2, 128, 2, 2 * DIM], F8, isOutput=False)
    wdown8_e = nc.declare_dram_parameter("W_down8", [2, 128, 2, HID], F8, isOutput=False)
    wq8_e = nc.declare_dram_parameter("WqT8", [128, 2, HID], F8, isOutput=False)
    wk8_e = nc.declare_dram_parameter("WkT8", [128, 2, HID], F8, isOutput=False)
    wv8_e = nc.declare_dram_parameter("WvT8", [128, 2, HID], F8, isOutput=False)
    wo_e = nc.declare_dram_parameter("WoT", [2, 128, HID], BF16, isOutput=False)
    wend8_e = nc.declare_dram_parameter("W_end8", [2, 128, 2, DIM], F8, isOutput=False)
    wcomb_e = nc.declare_dram_parameter("W_comb", [2, 128, DIM], BF16, isOutput=False)
    blin_e = nc.declare_dram_parameter("b_lin_pg", [128, 8], F32, isOutput=False)
    bdown_e = nc.declare_dram_parameter("b_down_pg", [128, 2], F32, isOutput=False)
    bend_e = nc.declare_dram_parameter("b_end_pg", [128, 4], F32, isOutput=False)
    gamma_e = nc.declare_dram_parameter("gamma_pg", [128, 4], F32, isOutput=False)
    beta_e = nc.declare_dram_parameter("beta_pg", [128, 4], F32, isOutput=False)
    temp_e = nc.declare_dram_parameter("lntau_col", [128, 2], F32, isOutput=False)
    ones_e = nc.declare_dram_parameter("ones_in", [128, 128], BF16, isOutput=False)
    ident_e = nc.declare_dram_parameter("ident_in", [128, 128], BF16, isOutput=False)
    id64_e = nc.declare_dram_parameter("id64_in", [128, 128], BF16, isOutput=False)
    id8_e = nc.declare_dram_parameter("id8_in", [128, 128], F8, isOutput=False)

    with tile.TileContext(nc) as tc, ExitStack() as ctx:
        wts = ctx.enter_context(tc.tile_pool(name="wts", bufs=1))
        xload = ctx.enter_context(tc.tile_pool(name="xload", bufs=3))
        u1p = ctx.enter_context(tc.tile_pool(name="u1p", bufs=1))
        rp = ctx.enter_context(tc.tile_pool(name="rp", bufs=2))
        u2p = ctx.enter_context(tc.tile_pool(name="u2p", bufs=1))
        ap_ = ctx.enter_context(tc.tile_pool(name="ap", bufs=2))
        bqp = ctx.enter_context(tc.tile_pool(name="bqp", bufs=1))
        qkvp = ctx.enter_context(tc.tile_pool(name="qkvp", bufs=2))
        sqp = ctx.enter_context(tc.tile_pool(name="sqp", bufs=2))
        nrmp = ctx.enter_context(tc.tile_pool(name="nrmp", bufs=2))
        qtp = ctx.enter_context(tc.tile_pool(name="qtp", bufs=2))
        etp = ctx.enter_context(tc.tile_pool(name="etp", bufs=3))
        rp2 = ctx.enter_context(tc.tile_pool(name="rp2", bufs=2))
        avp = ctx.enter_context(tc.tile_pool(name="avp", bufs=1))
        o1p = ctx.enter_context(tc.tile_pool(name="o1p", bufs=1))
        scr = ctx.enter_context(tc.tile_pool(name="scr", bufs=2))
        prep = ctx.enter_context(tc.tile_pool(name="prep", bufs=1))
        statp = ctx.enter_context(tc.tile_pool(name="statp", bufs=1))
        outp = ctx.enter_context(tc.tile_pool(name="outp", bufs=1))
        ps = ctx.enter_context(tc.tile_pool(name="ps", bufs=2, space="PSUM"))
        pst = ctx.enter_context(tc.tile_pool(name="pst", bufs=2, space="PSUM"))
        psg = ctx.enter_context(tc.tile_pool(name="psg", bufs=2, space="PSUM"))
        psav = ctx.enter_context(tc.tile_pool(name="psav", bufs=2, space="PSUM"))

        # ---- weights / constants in SBUF ----
        wlin8_sb = wts.tile([128, 2, 2, 2 * DIM], F8)
        wdown8_sb = wts.tile([128, 2, 2, HID], F8)
        wq8_sb = wts.tile([128, 2, HID], F8)
        wk8_sb = wts.tile([128, 2, HID], F8)
        wv8_sb = wts.tile([128, 2, HID], F8)
        wo_sb = wts.tile([128, 2, HID], BF16)
        wend8_sb = wts.tile([128, 2, 2, DIM], F8)
        wcomb_sb = wts.tile([128, 2, DIM], BF16)

        blin_sb = wts.tile([128, 8], F32)
        bdown_sb = wts.tile([128, 2], F32)
        bend_sb = wts.tile([128, 4], F32)
        gamma_sb = wts.tile([128, 4], F32)
        beta_sb = wts.tile([128, 4], F32)
        tempc_sb = wts.tile([128, 2], F32)
        ones_sb = wts.tile([128, 128], BF16)
        ident_sb = wts.tile([128, 128], BF16)
        id64_sb = wts.tile([128, 128], BF16)
        id8_sb = wts.tile([128, 128], F8)
        tiny_sb = wts.tile([128, 1], F32)
        nc.vector.memset(tiny_sb[:], TINY)
        epsln_sb = wts.tile([128, 1], F32)
        nc.vector.memset(epsln_sb[:], EPS_LN)
        zero_sb = wts.tile([128, 1], F32)
        nc.vector.memset(zero_sb[:], 0.0)

        def emit_loads(j):
            x1t8 = xload.tile([128, 4, 512], F8, tag="x1l8")
            x2t8 = xload.tile([128, 4, 512], F8, tag="x2l8")
            x1t = xload.tile([128, 4, 512], BF16, tag="x1l")
            nc.sync.dma_start(x1t8[:], x18_e[j].rearrange("k p t -> p k t"))
            nc.sync.dma_start(x2t8[:], x28_e[j].rearrange("k p t -> p k t"))
            nc.sync.dma_start(x1t[:], x1_e[j].rearrange("k p t -> p k t"))
            return (x1t, x1t8, x2t8)

        def emit_front_a(j, ld):
            x1t, x1t8, x2t8 = ld
            # ---- S1: h1 = relu((W_lin8^T X1_8)/SC + b); r = y1 + u1 (fp8) ----
            u1 = u1p.tile([128, 4, 512], F8)
            r = rp.tile([128, 4, 512], F8)
            for m in [4, 5, 6, 7, 0, 1, 2, 3]:
                pt = ps.tile([128, 512], F32, tag="ps")
                for kt in range(2):
                    nc.tensor.matmul(pt[:], wlin8_sb[:, kt, :, 128 * m:128 * (m + 1)],
                                     x1t8[:, 2 * kt:2 * kt + 2, :],
                                     start=(kt == 0), stop=(kt == 1),
                                     perf_mode=mybir.MatmulPerfMode.DoubleRow)
                if m >= 4:
                    nc.scalar.activation(u1[:, m - 4, :], pt[:], AF.Relu,
                                         bias=blin_sb[:, m:m + 1], scale=1.0 / SC)
                else:
                    ytmp = scr.tile([128, 512], BF16, tag="ytmp")
                    nc.scalar.activation(ytmp[:], pt[:], AF.Relu,
                                         bias=blin_sb[:, m:m + 1], scale=1.0 / SC)
                    nc.gpsimd.tensor_add(r[:, m, :], ytmp[:], u1[:, m, :])
            return dict(x1t=x1t, x2t8=x2t8, u1=u1, r=r)

        def emit_front_s1b(j, st):
            x2t8 = st["x2t8"]
            # ---- S1b: u2 = relu(W_lin[:,512:]^T X2 + b2) ----
            u2 = u2p.tile([128, 4, 512], F8)
            for m in range(4):
                pt = ps.tile([128, 512], F32, tag="ps")
                for kt in range(2):
                    nc.tensor.matmul(
                        pt[:],
                        wlin8_sb[:, kt, :, 512 + 128 * m:512 + 128 * (m + 1)],
                        x2t8[:, 2 * kt:2 * kt + 2, :],
                        start=(kt == 0), stop=(kt == 1),
                        perf_mode=mybir.MatmulPerfMode.DoubleRow)
                nc.scalar.activation(u2[:, m, :], pt[:], AF.Relu,
                                     bias=blin_sb[:, 4 + m:5 + m], scale=1.0 / SC)
            st["u2"] = u2

        def emit_front_b(j, st):
            u1, u2 = st["u1"], st["u2"]
            bbs = _bbs(j)
            nb = len(bbs)
            w_ = bbs[0][1]

            # ---- S2: A = relu(W_down^T u1 + b_down); Bq likewise from u2 ----
            A = ap_.tile([128, 2, 512], F8)
            Bq = bqp.tile([128, 2, 512], F8)
            for (dst, src) in ((A, u1), (Bq, u2)):
                for m in range(2):
                    pt = ps.tile([128, 512], F32, tag="ps")
                    for kt in range(2):
                        nc.tensor.matmul(pt[:],
                                         wdown8_sb[:, kt, :, 128 * m:128 * (m + 1)],
                                         src[:, 2 * kt:2 * kt + 2, :],
                                         start=(kt == 0), stop=(kt == 1),
                                         perf_mode=mybir.MatmulPerfMode.DoubleRow)
                    nc.scalar.activation(dst[:, m, :], pt[:], AF.Relu,
                                         bias=bdown_sb[:, m:m + 1], scale=1.0 / SC)

            # ---- S3: q = Wq@Bq, k = Wk@A (channel-major, SBUF copies);
            #          v -> v_ext with a ones column per branch segment ----
            q = qkvp.tile([128, 2, 512], BF16, tag="q")
            k = qkvp.tile([128, 2, 512], BF16, tag="k")
            vx = qkvp.tile([128, 2, VEXT], BF16, tag="vx")
            for (dst, w_sb, src) in ((q, wq8_sb, Bq), (k, wk8_sb, A)):
                for m in range(2):
                    pt = ps.tile([128, 512], F32, tag="ps")
                    nc.tensor.matmul(pt[:], w_sb[:, :, 128 * m:128 * (m + 1)],
                                     src[:, 0:2, :], start=True, stop=True,
                                     perf_mode=mybir.MatmulPerfMode.DoubleRow)
                    nc.vector.tensor_scalar_mul(dst[:, m, :], in0=pt[:],
                                                scalar1=1.0 / SC)
            for m in range(2):
                pt = ps.tile([128, 512], F32, tag="ps")
                nc.tensor.matmul(pt[:], wv8_sb[:, :, 128 * m:128 * (m + 1)],
                                 A[:, 0:2, :], start=True, stop=True,
                                 perf_mode=mybir.MatmulPerfMode.DoubleRow)
                vxg = vx[:, m, 0:nb * (w_ + 1)].rearrange(
                    "p (n e) -> p n e", e=w_ + 1)
                nc.vector.memset(vxg[:, :, w_:w_ + 1], 1.0)
                nc.vector.tensor_scalar_mul(
                    vxg[:, :, 0:w_],
                    in0=pt[:].rearrange("p (n w) -> p n w", w=w_),
                    scalar1=1.0 / SC)
            # kT transpose needs no norm scaling: do it as soon as k lands
            kT = qtp.tile([128, 4, 256], BF16, tag="kT")
            for tb in range(4):
                pt = pst.tile([128, 256], BF16, tag="pst")
                for g in range(2):
                    nc.tensor.matmul(
                        pt[:, 128 * g:128 * (g + 1)],
                        k[:, g, 128 * tb:128 * (tb + 1)], ident_sb[:],
                        is_transpose=True, start=(g == 0), stop=(g == 1))
                nc.vector.tensor_copy(kT[:, tb, :], pt[:])
            return dict(A=A, q=q, k=k, vx=vx, kT=kT)

        def emit_back_a(j, st):
            bbs = _bbs(j)
            nb = len(bbs)
            q, k = st["q"], st["k"]
            # ---- S4: rsqrt(row L2 norms) = exp(-0.5 ln(ssq)) ----
            w = 512 // nb
            rn = {}
            for (name, t_) in (("q", q), ("k", k)):
                sq = sqp.tile([128, 2, 512], BF16, tag="sq")
                nc.vector.tensor_mul(sq[:], t_[:], t_[:])
                ssq = nrmp.tile([128, 2, nb], F32, tag="ssq" + name)
                nc.vector.reduce_sum(
                    ssq[:], sq[:].rearrange("p g (n w) -> p g n w", w=w), axis=AX.X)
                lnt = nrmp.tile([128, 2, nb], F32, tag="ln" + name)
                nc.scalar.activation(lnt[:], ssq[:], AF.Ln, bias=tiny_sb[:, 0:1])
                rr = nrmp.tile([128, 2, nb], F32, tag="rn" + name)
                if name == "q":  # fold per-head temperature: exp(-ln(ssq)/2 + ln tau)
                    for g in range(2):
                        nc.scalar.activation(rr[:, g, :], lnt[:, g, :], AF.Exp,
                                             scale=-0.5,
                                             bias=tempc_sb[:, g:g + 1])
                else:
                    nc.scalar.activation(rr[:], lnt[:], AF.Exp, scale=-0.5)
                rn[name] = rr
            for g in range(2):
                for bi, (off, w_) in enumerate(bbs):
                    nc.vector.tensor_scalar_mul(
                        q[:, g, off:off + w_], in0=q[:, g, off:off + w_],
                        scalar1=rn["q"][:, g, bi:bi + 1])
            st["rnk"] = rn["k"]

        def emit_back_attn(j, st):
            bbs = _bbs(j)
            nb = len(bbs)
            q, vx, rnk, kT = st["q"], st["vx"], st["rnk"], st["kT"]
            # ---- S5: PE-transpose scaled q -> token-major qT ----
            qT = qtp.tile([128, 4, 256], BF16, tag="qT")
            for tb in range(4):
                pt = pst.tile([128, 256], BF16, tag="pst")
                for g in range(2):
                    nc.tensor.matmul(
                        pt[:, 128 * g:128 * (g + 1)],
                        q[:, g, 128 * tb:128 * (tb + 1)], ident_sb[:],
                        is_transpose=True, start=(g == 0), stop=(g == 1))
                nc.vector.tensor_copy(qT[:, tb, :], pt[:])

            # ---- S6-S8: per (group, branch): G^T -> exp(scale=rn_k) ->
            #      AV matmul with interleaved ones col -> R=1/S -> scale.
            #      After group 0, start the kt=0 half of S9's Wo matmul. ----
            av = avp.tile([128, 2, 512], BF16)
            for g in range(2):
                for bi, (off, w_) in enumerate(bbs):
                    chunks = _chunks(off, w_)
                    gps = psg.tile([128, 128], F32, tag="gps")
                    for ci, (tb, tpo, cw) in enumerate(chunks):
                        nc.tensor.matmul(
                            gps[:],
                            kT[tpo:tpo + cw, tb, 128 * g:128 * (g + 1)],
                            qT[tpo:tpo + cw, tb, 128 * g:128 * (g + 1)],
                            start=(ci == 0), stop=(ci == len(chunks) - 1))
                    ET = etp.tile([128, 128], BF16, tag="et")
                    nc.scalar.activation(ET[:], gps[:], AF.Exp,
                                         bias=zero_sb[:, 0:1],
                                         scale=rnk[:, g, bi:bi + 1])
                    pav = psav.tile([128, 512], F32, tag="pav")
                    e1 = w_ + 1
                    for h in range(4):
                        hs = slice(32 * h, 32 * (h + 1))
                        nc.tensor.matmul(
                            pav[hs, 0:e1], ET[hs, hs],
                            vx[hs, g, bi * e1:(bi + 1) * e1],
                            start=True, stop=True, tile_position=(32 * h, 32 * h))
                    R = rp2.tile([128, 1], F32, tag="R")
                    nc.vector.reciprocal_approx_fast(R[:], pav[:, w_:w_ + 1])
                    nc.scalar.mul(av[:, g, off:off + w_], pav[:, 0:w_],
                                  mul=R[:, 0:1])

            st["av"] = av

        def emit_back_tail(j, st):
            x1t, r, A, av = st["x1t"], st["r"], st["A"], st["av"]
            # ---- S9: o1 = Wo@av + A (A added via identity matmul) ----
            o1 = o1p.tile([128, 2, 512], BF16)
            for m in range(2):
                pt = ps.tile([128, 512], F32, tag="ps")
                for kt in range(2):
                    nc.tensor.matmul(pt[:], wo_sb[:, kt, 128 * m:128 * (m + 1)],
                                     av[:, kt, :], start=(kt == 0), stop=False)
                nc.tensor.matmul(pt[:], ident_sb[:], A[:, m, :],
                                 start=False, stop=True)
                nc.vector.tensor_copy(o1[:, m, :], pt[:])

            # ---- S10/S11: pre = W_end^T r + W_comb^T o1 + t1 + b_eff ----
            pre = prep.tile([128, 4, 512], BF16)
            s1ps = psav.tile([128, 512], F32, tag="pav")
            s2ps = psav.tile([128, 512], F32, tag="pav")
            for m in range(4):
                pt = ps.tile([128, 512], F32, tag="ps")
                for kt in range(2):
                    nc.tensor.matmul(pt[:], wend8_sb[:, kt, :, 128 * m:128 * (m + 1)],
                                     r[:, 2 * kt:2 * kt + 2, :],
                                     start=(kt == 0), stop=False,
                                     perf_mode=mybir.MatmulPerfMode.DoubleRow)
                for kt in range(2):
                    nc.tensor.matmul(pt[:], wcomb_sb[:, kt, 128 * m:128 * (m + 1)],
                                     o1[:, kt, :], start=False, stop=False)
                nc.tensor.matmul(pt[:], id64_sb[:], x1t[:, m, :],
                                 start=False, stop=True)
                nc.vector.tensor_scalar(pre[:, m, :], in0=pt[:],
                                        scalar1=1.0 / SC,
                                        scalar2=bend_sb[:, m:m + 1],
                                        op0=OP.mult, op1=OP.add)
                p2 = scr.tile([128, 512], BF16, tag="p2")
                nc.scalar.activation(p2[:], pre[:, m, :], AF.Square)
                nc.tensor.matmul(s1ps[:], ones_sb[:], pre[:, m, :],
                                 start=(m == 0), stop=(m == 3))
                nc.tensor.matmul(s2ps[:], ones_sb[:], p2[:],
                                 start=(m == 0), stop=(m == 3))

            # ---- S12: mu/rstd (rows replicated); rstd = exp(-0.5 ln(var)) ----
            mu = statp.tile([128, 512], BF16, tag="mu")
            nc.vector.tensor_scalar_mul(mu[:], in0=s1ps[:], scalar1=1.0 / DIM)
            msq = statp.tile([128, 512], BF16, tag="msq")
            nc.vector.tensor_mul(msq[:], mu[:], mu[:])
            var = statp.tile([128, 512], F32, tag="var")
            nc.vector.scalar_tensor_tensor(var[:], in0=s2ps[:], scalar=1.0 / DIM,
                                           in1=msq[:], op0=OP.mult, op1=OP.subtract)
            lnv = statp.tile([128, 512], F32, tag="lnv")
            nc.scalar.activation(lnv[:], var[:], AF.Ln, bias=epsln_sb[:, 0:1])
            rstd = statp.tile([128, 512], BF16, tag="rstd")
            nc.scalar.activation(rstd[:], lnv[:], AF.Exp, scale=-0.5)

            # ---- S13: out = ((pre - mu) * rstd) * gamma + beta ----
            ot = outp.tile([128, 4, 512], BF16)
            for m in range(4):
                t1 = scr.tile([128, 512], BF16, tag="t1")
                nc.gpsimd.tensor_sub(t1[:], pre[:, m, :], mu[:])
                mgb = scr.tile([128, 512], BF16, tag="mgb")
                nc.vector.tensor_mul(mgb[:], t1[:], rstd[:])
                nc.vector.tensor_scalar(
                    ot[:, m, :], in0=mgb[:], scalar1=gamma_sb[:, m:m + 1],
                    scalar2=beta_sb[:, m:m + 1], op0=OP.mult, op1=OP.add)

            # ---- S14: store ----
            nc.sync.dma_start(out_e[j].rearrange("k p t -> p k t"), ot[:])

        order = [1, 2, 0, 3, 4]
        nc.scalar.dma_start(wlin8_sb[:, 0], wlin8_e[0].rearrange("p o m -> p o m"))
        nc.scalar.dma_start(blin_sb[:], blin_e[:, :])
        nc.gpsimd.dma_start(wlin8_sb[:, 1], wlin8_e[1].rearrange("p o m -> p o m"))
        ld = emit_loads(order[0])
        nc.gpsimd.dma_start(wdown8_sb[:], wdown8_e.rearrange("k p o m -> p k o m"))
        nc.sync.dma_start(wq8_sb[:], wq8_e[:, :, :])
        nc.sync.dma_start(wk8_sb[:], wk8_e[:, :, :])
        nc.sync.dma_start(wv8_sb[:], wv8_e[:, :, :])
        nc.sync.dma_start(wo_sb[:], wo_e.rearrange("k p m -> p k m"))
        nc.sync.dma_start(wend8_sb[:], wend8_e.rearrange("k p o m -> p k o m"))
        nc.sync.dma_start(wcomb_sb[:], wcomb_e.rearrange("k p m -> p k m"))
        nc.sync.dma_start(bdown_sb[:], bdown_e[:, :])
        nc.sync.dma_start(bend_sb[:], bend_e[:, :])
        nc.sync.dma_start(gamma_sb[:], gamma_e[:, :])
        nc.sync.dma_start(beta_sb[:], beta_e[:, :])
        nc.sync.dma_start(tempc_sb[:], temp_e[:, :])
        nc.sync.dma_start(ones_sb[:], ones_e[:, :])
        nc.sync.dma_start(ident_sb[:], ident_e[:, :])
        nc.sync.dma_start(id64_sb[:], id64_e[:, :])
        nc.sync.dma_start(id8_sb[:], id8_e[:, :])

        st = emit_front_a(order[0], ld)
        emit_front_s1b(order[0], st)
        st.update(emit_front_b(order[0], st))
        prev = (order[0], st)
        for j in order[1:]:
            ld = emit_loads(j)
            emit_back_a(prev[0], prev[1])
            st = emit_front_a(j, ld)
            emit_front_s1b(j, st)
            emit_back_attn(prev[0], prev[1])
            st.update(emit_front_b(j, st))
            emit_back_tail(prev[0], prev[1])
            prev = (j, st)
        emit_back_a(prev[0], prev[1])
        emit_back_attn(prev[0], prev[1])
        emit_back_tail(prev[0], prev[1])

    # Run the act-table insertion pass with a curated set list so every
    # activation (relu/ln/exp/square/copy) resolves to the one combined
    # natural_log_exp_and_others set -> a single ACT_TABLE_LOAD.
    shared = {AF.Exp, AF.Ln, AF.Relu, AF.Square, AF.Copy, AF.Identity}
    tabs = get_activation_tables(nc.m.arch)
    curated = []
    for name, fns in tabs.items():
        if name != "natural_log_exp_and_others":
            fns = fns - shared
        curated.append((name, fns))

    orig = bacc.Bacc.insert_act_table_loads

    def _curated(self):
        _bass_rust.insert_act_table_loads(self, curated)

    nc.insert_act_table_loads = _curated.__get__(nc)
    try:
        nc.compile()
    finally:
        nc.insert_act_table_loads = orig.__get__(nc)
    return nc


# ---------------- host side ----------------
_CACHE = {}


def _get_runner():
    if "runner" in _CACHE:
        return _CACHE["runner"]
    import jax
    from jax.sharding import Mesh, PartitionSpec
    from jax.experimental.shard_map import shard_map
    from concourse.bass2jax import (
        _bass_exec_p, install_neuronx_cc_hook, partition_id_tensor)
    import concourse.mybir as mybir_

    nc = build_nc()
    install_neuronx_cc_hook()
    partition_name = nc.partition_id_tensor.name if nc.partition_id_tensor else None
    in_names, out_names, out_avals, zero_outs = [], [], [], []
    for alloc in nc.m.functions[0].allocations:
        if not isinstance(alloc, mybir_.MemoryLocationSet):
            continue
        name = alloc.memorylocations[0].name
        if alloc.kind == "ExternalInput":
            if name != partition_name:
                in_names.append(name)
        elif alloc.kind == "ExternalOutput":
            out_names.append(name)
            shape = tuple(alloc.tensor_shape)
            dtype = mybir_.dt.np(alloc.dtype)
            out_avals.append(jax.core.ShapedArray(shape, dtype))
            zero_outs.append(np.zeros(shape, dtype))
    n_params, n_outs = len(in_names), len(out_avals)
    all_in = list(in_names) + list(out_names)
    if partition_name is not None:
        all_in.append(partition_name)
    donate = tuple(range(n_params, n_params + n_outs))

    def _body(*args):
        operands = list(args)
        if partition_name is not None:
            operands.append(partition_id_tensor())
        return tuple(_bass_exec_p.bind(
            *operands, out_avals=tuple(out_avals), in_names=tuple(all_in),
            out_names=tuple(out_names), lowering_input_output_aliases=(),
            sim_require_finite=True, sim_require_nnan=True, nc=nc))

    devices = jax.devices()[:8]
    mesh = Mesh(np.asarray(devices), ("core",))
    fn = jax.jit(
        shard_map(_body, mesh=mesh,
                  in_specs=(PartitionSpec("core"),) * (n_params + n_outs),
                  out_specs=(PartitionSpec("core"),) * n_outs,
                  check_rep=False),
        donate_argnums=donate, keep_unused=True)
    _CACHE["runner"] = (fn, in_names, out_names, out_avals, zero_outs)
    return _CACHE["runner"]


def _reorder_x(xc, BF):
    """[8, 512, 320] fp32 -> [5, 4, 128, 512] bf16, token-reordered."""
    dev = np.empty((NT, 4, 128, 512), dtype=BF)
    z = np.transpose(xc[:, :, 0:64], (1, 0, 2)).reshape(512, 512)
    dev[0] = z.reshape(4, 128, 512).astype(BF)
    for j in range(1, NT):
        xx = np.transpose(xc[2 * j - 2:2 * j, :, 64:320], (1, 0, 2))
        dev[j] = xx.reshape(512, 512).reshape(4, 128, 512).astype(BF)
    return dev


def _unreorder_out(dev):
    """[5, 4, 128, 512] (any float) -> [8, 512, 320] fp32."""
    out = np.empty((B_LOC, DIM, NTOK), np.float32)
    z = dev[0].astype(np.float32).reshape(512, 8, 64)
    out[:, :, 0:64] = np.transpose(z, (1, 0, 2))
    for j in range(1, NT):
        xx = dev[j].astype(np.float32).reshape(512, 2, 256)
        out[2 * j - 2:2 * j, :, 64:320] = np.transpose(xx, (1, 0, 2))
    return out


def _prep_inputs(inputs):
    import ml_dtypes
    BF = ml_dtypes.bfloat16
    f = lambda a: np.ascontiguousarray(np.asarray(a), dtype=np.float32)
    x1 = f(inputs["x1"]).reshape(64, DIM, NTOK)
    x2 = f(inputs["x2"]).reshape(64, DIM, NTOK)
    temp = np.log(f(inputs["temperature"]).reshape(HEADS))
    temp_col = np.empty((128, 2), np.float32)
    for g in range(2):
        for hh in range(4):
            temp_col[32 * hh:32 * (hh + 1), g] = temp[4 * g + hh]

    F8N = ml_dtypes.float8_e4m3
    W_up = f(inputs["W_up"])
    W_end = f(inputs["W_end"])
    W_comb = ((W_up @ W_end) * 64.0).astype(BF)
    bend_eff = f(inputs["b_end"]) + f(inputs["b_up"]) @ W_end

    def dr8(w, n_out):  # [512, n_out] -> [kt'=2, ki=128, ko=2, n_out] e4m3 x64
        w8 = (w * 64.0).astype(F8N)
        return np.ascontiguousarray(
            w8.reshape(2, 2, 128, n_out).transpose(0, 2, 1, 3))

    def w8t(w):  # [256, 256] -> transposed [ki=128, ko=2, 256] e4m3 x64
        w8 = (f(w).T * 64.0).astype(F8N)
        return np.ascontiguousarray(w8.reshape(2, 128, HID).transpose(1, 0, 2))

    col = lambda b, n: np.ascontiguousarray(f(b).reshape(n, 128).T)
    shared = {
        "W_lin8": dr8(f(inputs["W_lin"]), 2 * DIM),
        "W_down8": dr8(f(inputs["W_down"]), HID),
        "WqT8": w8t(inputs["Wq"]),
        "WkT8": w8t(inputs["Wk"]),
        "WvT8": w8t(inputs["Wv"]),
        "WoT": np.ascontiguousarray(f(inputs["Wo"]).T).astype(BF).reshape(2, 128, HID),
        "W_end8": dr8(W_end, DIM),
        "W_comb": np.ascontiguousarray(W_comb).reshape(2, 128, DIM),
        "b_lin_pg": col(inputs["b_lin"], 8),
        "b_down_pg": col(inputs["b_down"], 2),
        "b_end_pg": np.ascontiguousarray(bend_eff.reshape(4, 128).T),
        "gamma_pg": col(inputs["gamma"], 4),
        "beta_pg": col(inputs["beta"], 4),
        "lntau_col": temp_col,
        "ones_in": np.ones((128, 128), BF),
        "ident_in": np.eye(128, dtype=np.float32).astype(BF),
        "id64_in": (64.0 * np.eye(128, dtype=np.float32)).astype(BF),
        "id8_in": np.eye(128, dtype=np.float32).astype(F8N),
    }
    in_maps = []
    for c in range(8):
        m = dict(shared)
        m["x1"] = _reorder_x(x1[8 * c:8 * (c + 1)], BF)
        m["x1_8"] = _reorder_x(x1[8 * c:8 * (c + 1)], F8N)
        m["x2_8"] = _reorder_x(x2[8 * c:8 * (c + 1)], F8N)
        in_maps.append(m)
    return in_maps


def run_in_maps(in_maps):
    """Run the prebuilt executable on 8 cores; returns per-core out arrays."""
    import jax
    fn, in_names, out_names, out_avals, zero_outs = _get_runner()
    per_core = [[np.asarray(m[name]) for name in in_names] for m in in_maps]
    concat_in = [np.concatenate([per_core[c][i] for c in range(8)], axis=0)
                 for i in range(len(in_names))]
    concat_zeros = [np.zeros((8 * z.shape[0], *z.shape[1:]), z.dtype)
                    for z in zero_outs]
    out = fn(*concat_in, *concat_zeros)
    jax.block_until_ready(out)
    oi = out_names.index("out")
    arr = np.asarray(out[oi]).reshape(8, *out_avals[oi].shape)
    return arr


def kernel(**inputs):
    in_maps = _prep_inputs(inputs)
    arr = run_in_maps(in_maps)  # [8, 5, 4, 128, 512] bf16
    full = np.empty((64, DIM, NTOK), np.float32)
    for c in range(8):
        full[8 * c:8 * (c + 1)] = _unreorder_out(arr[c])
    return full.reshape(64, DIM, 16, 20).astype(np.float32)


if __name__ == "__main__":
    rng = np.random.default_rng(0)
    ins = {
        "x1": rng.standard_normal((64, 512, 16, 20), dtype=np.float32),
        "x2": rng.standard_normal((64, 512, 16, 20), dtype=np.float32),
    }
    s = 0.02
    for nm, shape in [("W_lin", (512, 1024)), ("W_down", (512, 256)),
                      ("W_up", (256, 512)), ("Wq", (256, 256)),
                      ("Wk", (256, 256)), ("Wv", (256, 256)),
                      ("Wo", (256, 256)), ("W_end", (512, 512))]:
        ins[nm] = (rng.standard_normal(shape) * s).astype(np.float32)
    for nm, n in [("b_lin", 1024), ("b_down", 256), ("b_up", 512),
                  ("b_end", 512)]:
        ins[nm] = np.zeros(n, np.float32)
    ins["gamma"] = np.ones(512, np.float32)
    ins["beta"] = np.zeros(512, np.float32)
    ins["temperature"] = np.ones((8, 1, 1), np.float32)
    out = kernel(**ins)
    print("kernel ran, out shape", out.shape, "mean", float(np.abs(out).mean()))
